# revision 12
# baseline (speedup 1.0000x reference)
"""Trainium2 Bass kernel for nn_ClassEmbedding: embedding gather + tanh
feeding a 2-layer LSTM (hidden 512, T=8) over a fused batch of 12800,
data-parallel over 8 NeuronCores (1600 rows/core).

V3: step-major pass interleaving. The 1600 columns per core split into 4
passes of 400; the loop is for t: for p, so the 4 independent recurrence
chains hide each other's gate->h->gate dependency latency.

Layout: everything transposed. Gates are computed as
    gatesT[4R, B] = W_ihT-contract(xT) + W_hhT-contract(hT)
so hidden states live as hT/cT [512 -> 4x128 chunks, B] and the recurrence
needs zero transposes. Only the 300-dim embeddings are transposed (PE
transpose, 128-token tiles).

Precision: L1 runs fp8 DoubleRow at every step (its error is damped by
layer-2's gate slope). L2 runs fp8 DR for t < K8 and bf16 (weights
pre-scaled x512 so the PSUM scale matches the DR path) for t >= K8.
All K-parts of L1's x-contribution are DR: xa covers emb dims 0..255,
xb [23,2] covers dims 256..299 plus a bias row (value SA in the rhs,
64*b1 + fp8-residual in the lhsT) so b1 lands in PSUM and the layer-1
activations can run wide over chunk pairs with a plain scale=1/512.
Weights for the fp8 path are GPTQ-quantized on the host against
calibration activations from a small CPU reference run.
"""
import sys

sys.path.insert(0, "/opt/trn_rl_repo")

import numpy as np
import ml_dtypes

from concourse import bass, mybir
import concourse.tile as tile
from concourse.bass_utils import run_bass_kernel_spmd
from concourse.masks import make_identity
from concourse.vector_clock import ScopedClock

F32 = mybir.dt.float32
F16 = mybir.dt.float16
BF16 = mybir.dt.bfloat16
F8 = mybir.dt.float8e4
I32 = mybir.dt.int32
AFT = mybir.ActivationFunctionType
DRM = mybir.MatmulPerfMode.DoubleRow
NP8 = ml_dtypes.float8_e4m3
NPBF = ml_dtypes.bfloat16

P = 128
VOCAB, EMB, RNN, T = 20000, 300, 512, 8
B, NCLS = 64, 200
BN = B * NCLS            # 12800
NCORES = 8
BNC = BN // NCORES       # 1600 per core
PW = 400                 # pass width (batch columns per pass)
NPASS = BNC // PW        # 4
NM = 16                  # 2048 / 128 gate row chunks
TOKT = [(0, 128), (128, 128), (256, 128), (384, 16)]  # token tiles per pass

K8 = 6                   # steps t < K8 run layer-2 in fp8; the rest bf16
SW = 64.0                # fp8 weight scale
SA = 8.0                 # fp8 activation scale (products scaled SW*SA = 512)
DESC = 1.0 / (SW * SA)


def _patched_drain_and_barrier(self, tick_clock, wait_clock):
    # walrus rejects >2 sync waits on one instruction; spread the final
    # drain's waits across single-wait NOPs.
    nc = self.nc
    drain_inst = nc.sync.drain()
    wait_clock.add_sem_waits(
        drain_inst.ins, ScopedClock({None: tick_clock.global_clock})
    )
    si = drain_inst.ins.sync_info
    if si is not None and si.on_wait and len(si.on_wait) > 1:
        waits = list(si.on_wait)
        si.on_wait = waits[:1]
        for w in waits[1:]:
            nop = nc.sync.nop()
            nop.ins.sync_info = mybir.SyncInfo(on_wait=[w], on_update=[])
    nc.all_engine_barrier()
    assert self.sems is not None
    popped = nc._tile_sem_poison_stack.pop()
    assert popped is self._sem_poison
    nc.clear_and_free_semaphores(list(self.sems.allocated().values()))
    nc.all_engine_barrier()


tile.TileContext._drain_and_barrier = _patched_drain_and_barrier


def _split_waits(nc, maxw=1):
    """walrus rejects instructions carrying more than a couple of sync
    waits; keep at most `maxw` on each instruction and move the rest to
    preceding same-engine NOPs."""
    wid = 0
    for bb in nc.main_func.blocks:
        out = []
        changed = False
        for inst in bb.instructions:
            si = inst.sync_info
            if si is not None and si.on_wait and len(si.on_wait) > maxw:
                waits = list(si.on_wait)
                for w in waits[maxw:]:
                    nop = mybir.InstNoOp(name=f"wsplit-{wid}", ins=[], outs=[])
                    wid += 1
                    nop.engine = inst.engine
                    nop.sync_info = mybir.SyncInfo(on_wait=[w], on_update=[])
                    out.append(nop)
                inst.sync_info = mybir.SyncInfo(
                    on_wait=waits[:maxw], on_update=list(si.on_update or [])
                )
                changed = True
            out.append(inst)
        if changed:
            bb.instructions = out


def build_nc():
    nc = bass.Bass()
    w2v16 = nc.declare_dram_parameter("w2v16", [VOCAB, EMB], F16, isOutput=False)
    q11d_d = nc.declare_dram_parameter("q11d", [P, 2, 4 * RNN], F8, isOutput=False)
    q11r_d = nc.declare_dram_parameter("q11r", [23, 2, 4 * RNN], F8, isOutput=False)
    q1h_d = nc.declare_dram_parameter("q1h", [2, P, 2, 4 * RNN], F8, isOutput=False)
    q2i_d = nc.declare_dram_parameter("q2i", [2, P, 2, 4 * RNN], F8, isOutput=False)
    q2h_d = nc.declare_dram_parameter("q2h", [2, P, 2, 4 * RNN], F8, isOutput=False)
    w2ib_d = nc.declare_dram_parameter("w2ib", [RNN, 4 * RNN], BF16, isOutput=False)
    w2hb_d = nc.declare_dram_parameter("w2hb", [RNN, 4 * RNN], BF16, isOutput=False)
    b2d = nc.declare_dram_parameter("b2", [P, NM], F32, isOutput=False)
    idsd = nc.declare_dram_parameter("ids", [P, P], I32, isOutput=False)
    outd = nc.declare_dram_parameter("out", [RNN, BNC], F32, isOutput=True)

    with tile.TileContext(nc) as tc:
        with (
            tc.tile_pool(name="wp", bufs=1) as wp,
            tc.tile_pool(name="sp", bufs=1) as sp,
            tc.tile_pool(name="gb", bufs=10) as gb,
            tc.tile_pool(name="tp", bufs=6) as tp,
            tc.tile_pool(name="hb", bufs=2) as hb,
            tc.tile_pool(name="gp", bufs=3, space="PSUM") as gp,
            tc.tile_pool(name="tsp", bufs=2, space="PSUM") as tsp,
        ):
            # ---- small constants first: the sync DMA queue is FIFO, and
            # the gather pipeline only needs ids ----
            ids_sb = wp.tile([P, P], I32, name="ids_sb")
            nc.sync.dma_start(out=ids_sb[:], in_=idsd[:])
            b2_sb = wp.tile([P, NM], F32, name="b2_sb")
            nc.sync.dma_start(out=b2_sb[:], in_=b2d[:])
            ident32 = wp.tile([P, P], F32, name="ident32")
            make_identity(nc, ident32[:])
            ident = wp.tile([P, P], F16, name="ident")
            nc.vector.tensor_copy(out=ident[:], in_=ident32[:])

            # persistent embedding staging ring: 8 tiles = 2 slots of
            # gather prefetch so PE transposes never wait on the gather
            est_ring = [wp.tile([P, EMB], F16, name=f"est{i}") for i in range(8)]

            # ---- fp8 weights (needed first: step 0 is fp8) ----
            q11d = wp.tile([P, 2, 4 * RNN], F8, name="q11d")
            nc.sync.dma_start(out=q11d[:], in_=q11d_d[:])
            q11r = wp.tile([23, 2, 4 * RNN], F8, name="q11r")
            nc.sync.dma_start(out=q11r[:], in_=q11r_d[:])

            def load_dr_w(dram, label):
                chunks = []
                for i in range(2):
                    wt = wp.tile([P, 2, 4 * RNN], F8, name=f"{label}_{i}")
                    nc.sync.dma_start(out=wt[:], in_=dram[i])
                    chunks.append(wt)
                return chunks

            q1h = load_dr_w(q1h_d, "q1h")
            q2i = load_dr_w(q2i_d, "q2i")
            q2h = load_dr_w(q2h_d, "q2h")

            # ---- bf16 weights (layer-2 late steps), pre-scaled x512 ----
            def load_bf_w(dram, label):
                chunks = []
                for i in range(4):
                    wt = wp.tile([P, 4 * RNN], BF16, name=f"{label}_{i}")
                    nc.sync.dma_start(out=wt[:], in_=dram[i * P : (i + 1) * P, :])
                    chunks.append(wt)
                return chunks

            w2ib = load_bf_w(w2ib_d, "w2ib")
            w2hb = load_bf_w(w2hb_d, "w2hb")

            # ---- persistent state tiles (per pass) ----
            # fp8 DR layout: wide [128, 4, PW]; r-chunk r = (kb=r//2, j=r%2)
            # so the DR rhs for kb is tile[:, 2kb:2kb+2, :]
            h1d = [
                [sp.tile([P, 4, PW], F8, name=f"h1d_{bb}_{p_}") for p_ in range(NPASS)]
                for bb in range(2)
            ]
            h2d = [
                [sp.tile([P, 4, PW], F8, name=f"h2d_{bb}_{p_}") for p_ in range(NPASS)]
                for bb in range(2)
            ]
            # bf16 h2 for the tail steps (h2(t) read at t+1 >= K8)
            h2b = [sp.tile([P, 4, PW], BF16, name=f"h2b_{p_}") for p_ in range(NPASS)]
            c1 = [sp.tile([P, 4, PW], F16, name=f"c1_{p_}") for p_ in range(NPASS)]
            c2 = [sp.tile([P, 4, PW], F16, name=f"c2_{p_}") for p_ in range(NPASS)]
            # x double buffers (persistent so the xb bias row survives)
            xa_db = [sp.tile([P, 2, PW], F8, name=f"xa{i}") for i in range(2)]
            xb_db = [sp.tile([23, 2, PW], F8, name=f"xb{i}") for i in range(2)]
            for i in range(2):
                nc.vector.memset(xb_db[i][:], SA)

            def gen_gather(s, slot):
                """Issue the 4 indirect gathers for slot s into est ring
                half `slot` (0/1)."""
                ests = []
                for j, (to, tn) in enumerate(TOKT):
                    g = s * len(TOKT) + j
                    est = est_ring[slot * 4 + j]
                    nc.gpsimd.indirect_dma_start(
                        out=est[:tn, :],
                        out_offset=None,
                        in_=w2v16[:],
                        in_offset=bass.IndirectOffsetOnAxis(
                            ap=ids_sb[:tn, g : g + 1], axis=0
                        ),
                    )
                    ests.append(est)
                return ests

            def gen_x(ests, slot):
                """PE-transpose gathered [tokens, kchunk] tiles into the
                fp8 DR rhs tiles (xa: emb 0..255, xb: 256..299), applying
                the x8 activation scale during the PSUM->SBUF descale copy.
                Copies alternate vector/gpsimd to balance engines."""
                xa = xa_db[slot]
                xb = xb_db[slot]
                nv = 0
                for j, (to, tn) in enumerate(TOKT):
                    est = ests[j]
                    for c in range(2):
                        tpp = tsp.tile([P, 2, P], F16, name="tpp")
                        nc.tensor.transpose(
                            out=tpp[:, 0, :tn],
                            in_=est[:tn, c * P : (c + 1) * P],
                            identity=ident[:tn, :tn],
                        )
                        nc.vector.tensor_scalar_mul(
                            xa[:, c, to : to + tn], tpp[:, 0, :tn], SA
                        )
                    # 44 leftover dims as [22, 2]: (256+p, 278+p)
                    tpp2 = tsp.tile([P, 2, P], F16, name="tpp")
                    nc.tensor.transpose(
                        out=tpp2[:22, 0, :tn],
                        in_=est[:tn, 256:278],
                        identity=ident[:tn, :tn],
                    )
                    nc.tensor.transpose(
                        out=tpp2[:22, 1, :tn],
                        in_=est[:tn, 278:300],
                        identity=ident[:tn, :tn],
                    )
                    nc.vector.tensor_scalar_mul(
                        xb[:22, :, to : to + tn], tpp2[:22, :, :tn], SA
                    )
                return (xa, xb)

            def l1_layer(x_cur, p_, t):
                """Layer 1: fp8 DR matmuls + wide-2 activations (bias is
                folded into the xb pack). Returns 4 wide gate tiles
                [128, 4, PW] in order i, f, g, o (f None at t0)."""
                t0 = t == 0
                xa, xb = x_cur
                ga = [None] * 4
                for gt in range(4):          # gate type: i, f, g, o
                    if t0 and gt == 1:
                        continue
                    gw = gb.tile([P, 4, PW], F16, name="gt")
                    for half in range(2):    # chunk pair (4gt+2*half, +1)
                        ps = gp.tile([P, 2, 512], F32, name="ps")
                        for jj in range(2):
                            mi = 4 * gt + 2 * half + jj
                            dst = ps[:, jj, :PW]
                            nc.tensor.matmul(
                                dst,
                                lhsT=q11d[:, :, mi * P : (mi + 1) * P],
                                rhs=xa[:],
                                start=True,
                                stop=False,
                                perf_mode=DRM,
                            )
                            nc.tensor.matmul(
                                dst,
                                lhsT=q11r[:, :, mi * P : (mi + 1) * P],
                                rhs=xb[:],
                                start=False,
                                stop=t0,
                                perf_mode=DRM,
                            )
                            if not t0:
                                for kb in range(2):
                                    nc.tensor.matmul(
                                        dst,
                                        lhsT=q1h[kb][:, :, mi * P : (mi + 1) * P],
                                        rhs=h1d[(t - 1) % 2][p_][:, 2 * kb : 2 * kb + 2, :],
                                        start=False,
                                        stop=kb == 1,
                                        perf_mode=DRM,
                                    )
                        func = AFT.Tanh if gt == 2 else AFT.Sigmoid
                        nc.scalar.activation(
                            out=gw[:, 2 * half : 2 * half + 2, :],
                            in_=ps[:, :, :PW],
                            func=func,
                            scale=DESC,
                        )
                    ga[gt] = gw
                return ga

            def l2_layer(p_, t):
                """Layer 2: fp8 DR (t<K8) or bf16 (t>=K8) matmuls + narrow
                activations carrying the b2 bias."""
                t0 = t == 0
                fp8 = t < K8
                ga = [None] * 4
                for gt in range(4):
                    if t0 and gt == 1:
                        continue
                    gw = gb.tile([P, 4, PW], F16, name="gt")
                    for half in range(2):
                        ps = gp.tile([P, 2, 512], F32, name="ps")
                        for jj in range(2):
                            mi = 4 * gt + 2 * half + jj
                            dst = ps[:, jj, :PW]
                            first = True
                            if not t0:
                                if fp8:
                                    for kb in range(2):
                                        nc.tensor.matmul(
                                            dst,
                                            lhsT=q2h[kb][:, :, mi * P : (mi + 1) * P],
                                            rhs=h2d[(t - 1) % 2][p_][:, 2 * kb : 2 * kb + 2, :],
                                            start=first,
                                            stop=False,
                                            perf_mode=DRM,
                                        )
                                        first = False
                                else:
                                    for k in range(4):
                                        nc.tensor.matmul(
                                            dst,
                                            lhsT=w2hb[k][:, mi * P : (mi + 1) * P],
                                            rhs=h2b[p_][:, k, :],
                                            start=first,
                                            stop=False,
                                        )
                                        first = False
                            if fp8:
                                for kb in range(2):
                                    nc.tensor.matmul(
                                        dst,
                                        lhsT=q2i[kb][:, :, mi * P : (mi + 1) * P],
                                        rhs=h1d[t % 2][p_][:, 2 * kb : 2 * kb + 2, :],
                                        start=first,
                                        stop=kb == 1,
                                        perf_mode=DRM,
                                    )
                                    first = False
                            else:
                                h1w = h1b_cur[0]
                                for k in range(4):
                                    nc.tensor.matmul(
                                        dst,
                                        lhsT=w2ib[k][:, mi * P : (mi + 1) * P],
                                        rhs=h1w[:, k, :],
                                        start=first,
                                        stop=k == 3,
                                    )
                                    first = False
                        func = AFT.Tanh if gt == 2 else AFT.Sigmoid
                        for jj in range(2):
                            mi = 4 * gt + 2 * half + jj
                            nc.scalar.activation(
                                out=gw[:, 2 * half + jj, :],
                                in_=ps[:, jj, :PW],
                                func=func,
                                bias=b2_sb[:, mi : mi + 1],
                                scale=DESC,
                            )
                    ga[gt] = gw
                return ga

            def update(ga, c, t0, out8=None, outb=None, outf=None):
                """Wide c update + h writes. ga = [i, f, g, o] wide tiles.
                out8: fp8 DR tile [128,4,PW] (written as SA*h); outb: bf16
                wide tile; outf: fp32 wide tile."""
                gi, gf, gg, go = ga
                if t0:
                    nc.vector.tensor_mul(out=c[:], in0=gi[:], in1=gg[:])
                else:
                    p1 = tp.tile([P, 4, PW], F16, name="tpt")
                    nc.vector.tensor_mul(out=p1[:], in0=gf[:], in1=c[:])
                    p2 = tp.tile([P, 4, PW], F16, name="tpt")
                    nc.gpsimd.tensor_mul(out=p2[:], in0=gi[:], in1=gg[:])
                    nc.vector.tensor_add(out=c[:], in0=p1[:], in1=p2[:])
                th = tp.tile([P, 4, PW], F16, name="tpt")
                nc.scalar.activation(out=th[:], in_=c[:], func=AFT.Tanh)
                ndst = (out8 is not None) + (outb is not None) + (outf is not None)
                if ndst > 1:
                    hf = tp.tile([P, 4, PW], F16, name="tpt")
                    nc.vector.tensor_mul(out=hf[:], in0=go[:], in1=th[:])
                    if out8 is not None:
                        nc.gpsimd.tensor_scalar_mul(out8[:], hf[:], SA)
                    if outb is not None:
                        nc.vector.tensor_copy(out=outb[:], in_=hf[:])
                    if outf is not None:
                        nc.vector.tensor_copy(out=outf[:], in_=hf[:])
                elif out8 is not None:
                    # (go * SA) * th -> fp8 in one fused op
                    nc.vector.scalar_tensor_tensor(
                        out=out8[:], in0=go[:], scalar=SA, in1=th[:],
                        op0=mybir.AluOpType.mult, op1=mybir.AluOpType.mult,
                    )
                elif outb is not None:
                    nc.vector.tensor_mul(out=outb[:], in0=go[:], in1=th[:])
                else:
                    nc.vector.tensor_mul(out=outf[:], in0=go[:], in1=th[:])

            NS = T * NPASS
            ests = {0: gen_gather(0, 0), 1: gen_gather(1, 1)}
            x_cur = gen_x(ests.pop(0), 0)
            h1b_cur = [None]
            for t in range(T):
                for p_ in range(NPASS):
                    s = t * NPASS + p_
                    wb = t % 2
                    t0 = t == 0
                    tail = t >= K8
                    g1 = l1_layer(x_cur, p_, t)
                    # h1 destinations: fp8 for next-step L1 and fp8-L2;
                    # bf16 for the bf16 L2 path
                    o8 = h1d[wb][p_] if (t < T - 1 or not tail) else None
                    if tail:
                        h1b_cur[0] = hb.tile([P, 4, PW], BF16, name="h1b")
                    update(g1, c1[p_], t0, out8=o8,
                           outb=h1b_cur[0] if tail else None)
                    # prefetch: gathers two slots ahead, transposes one
                    # slot ahead (fills the PE gap before L2's h1 wait)
                    if s + 2 < NS:
                        ests[s + 2] = gen_gather(s + 2, (s + 2) % 2)
                    if s + 1 < NS:
                        x_next = gen_x(ests.pop(s + 1), (s + 1) % 2)
                    else:
                        x_next = None
                    g2 = l2_layer(p_, t)
                    o8 = h2d[wb][p_] if t + 1 < K8 else None
                    ob = h2b[p_] if K8 - 1 <= t < T - 1 else None
                    if t == T - 1:
                        of = hb.tile([P, 4, PW], F32, name="h2f")
                    else:
                        of = None
                    update(g2, c2[p_], t0, out8=o8, outb=ob, outf=of)
                    if t == T - 1:
                        for r in range(4):
                            nc.sync.dma_start(
                                out=outd[r * P : (r + 1) * P, p_ * PW : (p_ + 1) * PW],
                                in_=of[:, r, :],
                            )
                    x_cur = x_next
    _split_waits(nc)
    return nc


_NC_CACHE = None


def _get_nc():
    global _NC_CACHE
    if _NC_CACHE is None:
        _NC_CACHE = build_nc()
    return _NC_CACHE


def _sigmoid(x):
    return 1.0 / (1.0 + np.exp(-x))


def _gptq_e4m3(W, X, sc):
    """GPTQ-quantize W [M,K] to e4m3 at scale sc, calibrated on inputs
    X [N,K]. Returns the scaled quantized weights (fp32 values of sc*W)."""
    K = W.shape[1]
    H = (X.T @ X) / max(len(X), 1)
    H[np.diag_indices(K)] += 0.01 * np.mean(np.diag(H)) + 1e-8
    Hinv = np.linalg.inv(H)
    Wq = (W * sc).astype(np.float32).copy()
    Q = np.zeros_like(Wq)
    for j in range(K):
        q = Wq[:, j].astype(NP8).astype(np.float32)
        Q[:, j] = q
        err = (Wq[:, j] - q) / Hinv[j, j]
        if j + 1 < K:
            Wq[:, j + 1 :] -= np.outer(err, Hinv[j, j + 1 :])
    return Q


def _prep_core_inputs(sentence, word2vec, W_ih1, W_hh1, b_ih1, b_hh1,
                      W_ih2, W_hh2, b_ih2, b_hh2):
    f = lambda a: np.ascontiguousarray(np.asarray(a), dtype=np.float32)
    ids_all = np.asarray(sentence).reshape(BN, T).astype(np.int32)
    w2v = f(word2vec)
    w2v16 = np.tanh(w2v).astype(np.float16)
    W0 = {"W_ih1": f(W_ih1), "W_hh1": f(W_hh1),
          "W_ih2": f(W_ih2), "W_hh2": f(W_hh2)}
    b1 = f(b_ih1) + f(b_hh1)
    b2 = f(b_ih2) + f(b_hh2)

    # ---- bf16 weights (layer-2 late steps), pre-scaled x512 so the
    # PSUM scale matches the fp8 DR path ----
    fw = lambda a: np.ascontiguousarray((a.T * (SW * SA)).astype(NPBF))
    w2ib = fw(W0["W_ih2"])
    w2hb = fw(W0["W_hh2"])

    # ---- calibration run (CPU, fp32) for GPTQ Hessians ----
    CAL = 512
    cs = ids_all[:: max(BN // CAL, 1)][:CAL]
    h1 = np.zeros((CAL, RNN), np.float32)
    c1 = np.zeros_like(h1)
    h2 = np.zeros_like(h1)
    c2 = np.zeros_like(h1)
    Xx, Xh1, Xh1b, Xh2 = [], [], [], []
    for t in range(T):
        x = w2v16[cs[:, t]].astype(np.float32)
        Xx.append(x)
        Xh1.append(h1.copy())
        Xh2.append(h2.copy())
        g1 = x @ W0["W_ih1"].T + h1 @ W0["W_hh1"].T + b1
        i_, f_, g_, o_ = np.split(g1, 4, axis=1)
        c1 = _sigmoid(f_) * c1 + _sigmoid(i_) * np.tanh(g_)
        h1 = _sigmoid(o_) * np.tanh(c1)
        Xh1b.append(h1.copy())
        g2 = h1 @ W0["W_ih2"].T + h2 @ W0["W_hh2"].T + b2
        i_, f_, g_, o_ = np.split(g2, 4, axis=1)
        c2 = _sigmoid(f_) * c2 + _sigmoid(i_) * np.tanh(g_)
        h2 = _sigmoid(o_) * np.tanh(c2)

    Q = {}
    Q["W_ih1"] = _gptq_e4m3(W0["W_ih1"], np.concatenate(Xx) * SA, SW)
    Q["W_hh1"] = _gptq_e4m3(W0["W_hh1"], np.concatenate(Xh1) * SA, SW)
    Q["W_ih2"] = _gptq_e4m3(W0["W_ih2"], np.concatenate(Xh1b[:K8]) * SA, SW)
    Q["W_hh2"] = _gptq_e4m3(W0["W_hh2"], np.concatenate(Xh2[:K8]) * SA, SW)

    # ---- DR packs: tile[ki, j, m] = Q[m, 256*kb + 128*j + ki] ----
    def dr_pack(Qm, kb):
        lo = Qm[:, 256 * kb : 256 * kb + 128].T          # [128, M]
        hi = Qm[:, 256 * kb + 128 : 256 * kb + 256].T    # [128, M]
        return np.ascontiguousarray(
            np.stack([lo, hi], axis=1).astype(NP8))       # [128, 2, M]

    q11d = dr_pack(Q["W_ih1"], 0)
    # xb pack [23, 2, M]: rows p<22 = emb dims (256+p, 278+p); row 22 is
    # the bias (rhs value SA): j=0 main e4m3(64*b1), j=1 fp8 residual
    q11r = np.zeros((23, 2, 4 * RNN), np.float32)
    q11r[:22, 0, :] = Q["W_ih1"][:, 256:278].T
    q11r[:22, 1, :] = Q["W_ih1"][:, 278:300].T
    bmain = (SW * b1).astype(NP8).astype(np.float32)
    bres = (SW * b1 - bmain).astype(NP8).astype(np.float32)
    q11r[22, 0, :] = bmain
    q11r[22, 1, :] = bres
    q11r = np.ascontiguousarray(q11r.astype(NP8))
    q1h = np.stack([dr_pack(Q["W_hh1"], kb) for kb in range(2)])
    q2i = np.stack([dr_pack(Q["W_ih2"], kb) for kb in range(2)])
    q2h = np.stack([dr_pack(Q["W_hh2"], kb) for kb in range(2)])

    b2m = f(b2.reshape(NM, P).T)

    in_maps = []
    for k in range(NCORES):
        ids_k = ids_all[k * BNC : (k + 1) * BNC]
        ids_arr = np.zeros((P, P), dtype=np.int32)
        for t in range(T):
            for p_ in range(NPASS):
                s = t * NPASS + p_
                for j, (to, tn) in enumerate(TOKT):
                    g = s * len(TOKT) + j
                    ids_arr[:tn, g] = ids_k[p_ * PW + to : p_ * PW + to + tn, t]
        in_maps.append(
            {
                "w2v16": w2v16,
                "q11d": q11d,
                "q11r": q11r,
                "q1h": q1h,
                "q2i": q2i,
                "q2h": q2h,
                "w2ib": w2ib,
                "w2hb": w2hb,
                "b2": b2m,
                "ids": ids_arr,
            }
        )
    return in_maps


def kernel(sentence, word2vec, W_ih1, W_hh1, b_ih1, b_hh1,
           W_ih2, W_hh2, b_ih2, b_hh2, _trace=False, _return_perf=None):
    nc = _get_nc()
    in_maps = _prep_core_inputs(
        sentence, word2vec, W_ih1, W_hh1, b_ih1, b_hh1, W_ih2, W_hh2, b_ih2, b_hh2
    )
    res = run_bass_kernel_spmd(
        nc, in_maps, core_ids=list(range(NCORES)), trace=_trace
    )
    if _return_perf is not None:
        _return_perf.append(res)
    parts = [res.results[k]["out"].T for k in range(NCORES)]
    out = np.concatenate(parts, axis=0).reshape(B, NCLS, RNN)
    return np.ascontiguousarray(out, dtype=np.float32)


# revision 16
# speedup vs baseline: 1.1834x; 1.1834x over previous
"""Trainium2 Bass kernel for nn_ClassEmbedding: embedding gather + tanh
feeding a 2-layer LSTM (hidden 512, T=8) over a fused batch of 12800,
data-parallel over 8 NeuronCores (1600 rows/core).

V3: step-major pass interleaving. The 1600 columns per core split into 4
passes of 400; the loop is for t: for p, so the 4 independent recurrence
chains hide each other's gate->h->gate dependency latency.

Layout: everything transposed. Gates are computed as
    gatesT[4R, B] = W_ihT-contract(xT) + W_hhT-contract(hT)
so hidden states live as hT/cT [512 -> 4x128 chunks, B] and the recurrence
needs zero transposes. Only the 300-dim embeddings are transposed (PE
transpose, 128-token tiles).

Precision: L1 runs fp8 DoubleRow at every step (its error is damped by
layer-2's gate slope). L2 runs fp8 DR for t < K8 and bf16 (weights
pre-scaled x512 so the PSUM scale matches the DR path) for t >= K8.
All K-parts of L1's x-contribution are DR: xa covers emb dims 0..255,
xb [23,2] covers dims 256..299 plus a bias row (value SA in the rhs,
64*b1 + fp8-residual in the lhsT) so b1 lands in PSUM and the layer-1
activations can run wide over chunk pairs with a plain scale=1/512.
Weights for the fp8 path are GPTQ-quantized on the host against
calibration activations from a small CPU reference run.
"""
import sys

sys.path.insert(0, "/opt/trn_rl_repo")

import numpy as np
import ml_dtypes

from concourse import bass, mybir
import concourse.tile as tile
from concourse.bass_utils import run_bass_kernel_spmd
from concourse.masks import make_identity
from concourse.vector_clock import ScopedClock

F32 = mybir.dt.float32
F16 = mybir.dt.float16
BF16 = mybir.dt.bfloat16
F8 = mybir.dt.float8e4
I32 = mybir.dt.int32
AFT = mybir.ActivationFunctionType
DRM = mybir.MatmulPerfMode.DoubleRow
NP8 = ml_dtypes.float8_e4m3
NPBF = ml_dtypes.bfloat16

P = 128
VOCAB, EMB, RNN, T = 20000, 300, 512, 8
B, NCLS = 64, 200
BN = B * NCLS            # 12800
NCORES = 8
BNC = BN // NCORES       # 1600 per core
PW = 400                 # pass width (batch columns per pass)
NPASS = BNC // PW        # 4
NM = 16                  # 2048 / 128 gate row chunks
TOKT = [(0, 128), (128, 128), (256, 128), (384, 16)]  # token tiles per pass

K8 = 6                   # steps t < K8 run layer-2 in fp8; the rest bf16
SW = 64.0                # fp8 weight scale
SA = 8.0                 # fp8 activation scale (products scaled SW*SA = 512)
DESC = 1.0 / (SW * SA)


def _patched_drain_and_barrier(self, tick_clock, wait_clock):
    # walrus rejects >2 sync waits on one instruction; spread the final
    # drain's waits across single-wait NOPs.
    nc = self.nc
    drain_inst = nc.sync.drain()
    wait_clock.add_sem_waits(
        drain_inst.ins, ScopedClock({None: tick_clock.global_clock})
    )
    si = drain_inst.ins.sync_info
    if si is not None and si.on_wait and len(si.on_wait) > 1:
        waits = list(si.on_wait)
        si.on_wait = waits[:1]
        for w in waits[1:]:
            nop = nc.sync.nop()
            nop.ins.sync_info = mybir.SyncInfo(on_wait=[w], on_update=[])
    nc.all_engine_barrier()
    assert self.sems is not None
    popped = nc._tile_sem_poison_stack.pop()
    assert popped is self._sem_poison
    nc.clear_and_free_semaphores(list(self.sems.allocated().values()))
    nc.all_engine_barrier()


tile.TileContext._drain_and_barrier = _patched_drain_and_barrier


def _split_waits(nc, maxw=1):
    """walrus rejects instructions carrying more than a couple of sync
    waits; keep at most `maxw` on each instruction and move the rest to
    preceding same-engine NOPs."""
    wid = 0
    for bb in nc.main_func.blocks:
        out = []
        changed = False
        for inst in bb.instructions:
            si = inst.sync_info
            if si is not None and si.on_wait and len(si.on_wait) > maxw:
                waits = list(si.on_wait)
                for w in waits[maxw:]:
                    nop = mybir.InstNoOp(name=f"wsplit-{wid}", ins=[], outs=[])
                    wid += 1
                    nop.engine = inst.engine
                    nop.sync_info = mybir.SyncInfo(on_wait=[w], on_update=[])
                    out.append(nop)
                inst.sync_info = mybir.SyncInfo(
                    on_wait=waits[:maxw], on_update=list(si.on_update or [])
                )
                changed = True
            out.append(inst)
        if changed:
            bb.instructions = out


def build_nc():
    nc = bass.Bass()
    w2v16 = nc.declare_dram_parameter("w2v16", [VOCAB, EMB], F16, isOutput=False)
    q11d_d = nc.declare_dram_parameter("q11d", [P, 2, 4 * RNN], F8, isOutput=False)
    q11r_d = nc.declare_dram_parameter("q11r", [P, 2, 4 * RNN], F8, isOutput=False)
    q1h_d = nc.declare_dram_parameter("q1h", [2, P, 2, 4 * RNN], F8, isOutput=False)
    q2i_d = nc.declare_dram_parameter("q2i", [2, P, 2, 4 * RNN], F8, isOutput=False)
    q2h_d = nc.declare_dram_parameter("q2h", [2, P, 2, 4 * RNN], F8, isOutput=False)
    w2ib_d = nc.declare_dram_parameter("w2ib", [RNN, 4 * RNN], BF16, isOutput=False)
    w2hb_d = nc.declare_dram_parameter("w2hb", [RNN, 4 * RNN], BF16, isOutput=False)
    b2d = nc.declare_dram_parameter("b2", [P, NM], F32, isOutput=False)
    idsd = nc.declare_dram_parameter("ids", [P, P], I32, isOutput=False)
    xbc_d = nc.declare_dram_parameter("xbc", [P, 2, PW], F8, isOutput=False)
    outd = nc.declare_dram_parameter("out", [RNN, BNC], F32, isOutput=True)

    with tile.TileContext(nc) as tc:
        with (
            tc.tile_pool(name="wp", bufs=1) as wp,
            tc.tile_pool(name="sp", bufs=1) as sp,
            tc.tile_pool(name="gb", bufs=10) as gb,
            tc.tile_pool(name="tp", bufs=6) as tp,
            tc.tile_pool(name="hb", bufs=2) as hb,
            tc.tile_pool(name="gp", bufs=3, space="PSUM") as gp,
            tc.tile_pool(name="tsp", bufs=2, space="PSUM") as tsp,
        ):
            # ---- small constants first: the sync DMA queue is FIFO, and
            # the gather pipeline only needs ids ----
            ids_sb = wp.tile([P, P], I32, name="ids_sb")
            nc.sync.dma_start(out=ids_sb[:], in_=idsd[:])
            b2_sb = wp.tile([P, NM], F32, name="b2_sb")
            nc.sync.dma_start(out=b2_sb[:], in_=b2d[:])
            ident32 = wp.tile([P, P], F32, name="ident32")
            make_identity(nc, ident32[:])
            ident = wp.tile([P, P], F16, name="ident")
            nc.vector.tensor_copy(out=ident[:], in_=ident32[:])

            # persistent embedding staging ring: 8 tiles = 2 slots of
            # gather prefetch so PE transposes never wait on the gather
            est_ring = [wp.tile([P, EMB], F16, name=f"est{i}") for i in range(8)]

            # ---- fp8 weights (needed first: step 0 is fp8) ----
            q11d = wp.tile([P, 2, 4 * RNN], F8, name="q11d")
            nc.sync.dma_start(out=q11d[:], in_=q11d_d[:])
            q11r = wp.tile([P, 2, 4 * RNN], F8, name="q11r")
            nc.sync.dma_start(out=q11r[:], in_=q11r_d[:])

            def load_dr_w(dram, label):
                chunks = []
                for i in range(2):
                    wt = wp.tile([P, 2, 4 * RNN], F8, name=f"{label}_{i}")
                    nc.sync.dma_start(out=wt[:], in_=dram[i])
                    chunks.append(wt)
                return chunks

            q1h = load_dr_w(q1h_d, "q1h")
            q2i = load_dr_w(q2i_d, "q2i")
            q2h = load_dr_w(q2h_d, "q2h")

            # ---- bf16 weights (layer-2 late steps), pre-scaled x512 ----
            def load_bf_w(dram, label):
                chunks = []
                for i in range(4):
                    wt = wp.tile([P, 4 * RNN], BF16, name=f"{label}_{i}")
                    nc.sync.dma_start(out=wt[:], in_=dram[i * P : (i + 1) * P, :])
                    chunks.append(wt)
                return chunks

            w2ib = load_bf_w(w2ib_d, "w2ib")
            w2hb = load_bf_w(w2hb_d, "w2hb")

            # ---- persistent state tiles (per pass) ----
            # fp8 DR layout: wide [128, 4, PW]; r-chunk r = (kb=r//2, j=r%2)
            # so the DR rhs for kb is tile[:, 2kb:2kb+2, :]
            h1d = [
                [sp.tile([P, 4 * PW], F8, name=f"h1d_{bb}_{p_}") for p_ in range(NPASS)]
                for bb in range(2)
            ]
            h2d = [
                [sp.tile([P, 4 * PW], F8, name=f"h2d_{bb}_{p_}") for p_ in range(NPASS)]
                for bb in range(2)
            ]
            # bf16 h2 for the tail steps (h2(t) read at t+1 >= K8)
            h2b = [sp.tile([P, 4 * PW], BF16, name=f"h2b_{p_}") for p_ in range(NPASS)]
            c1 = [sp.tile([P, 4 * PW], F16, name=f"c1_{p_}") for p_ in range(NPASS)]
            c2 = [sp.tile([P, 4 * PW], F16, name=f"c2_{p_}") for p_ in range(NPASS)]
            # x double buffers (persistent so the xb bias row survives)
            xa_db = [sp.tile([P, 2, PW], F8, name=f"xa{i}") for i in range(2)]
            xb_db = [sp.tile([P, 2, PW], F8, name=f"xb{i}") for i in range(2)]
            for i in range(2):
                nc.sync.dma_start(out=xb_db[i][:], in_=xbc_d[:])


            def gen_gather(s, slot):
                """Issue the 4 indirect gathers for slot s into est ring
                half `slot` (0/1)."""
                ests = []
                for j, (to, tn) in enumerate(TOKT):
                    g = s * len(TOKT) + j
                    est = est_ring[slot * 4 + j]
                    nc.gpsimd.indirect_dma_start(
                        out=est[:tn, :],
                        out_offset=None,
                        in_=w2v16[:],
                        in_offset=bass.IndirectOffsetOnAxis(
                            ap=ids_sb[:tn, g : g + 1], axis=0
                        ),
                    )
                    ests.append(est)
                return ests

            def gen_x(ests, slot):
                """PE-transpose gathered [tokens, kchunk] tiles into the
                fp8 DR rhs tiles (xa: emb 0..255, xb: 256..299), applying
                the x8 activation scale during the PSUM->SBUF descale copy.
                Copies alternate vector/gpsimd to balance engines."""
                xa = xa_db[slot]
                xb = xb_db[slot]
                nv = 0
                for j, (to, tn) in enumerate(TOKT):
                    est = ests[j]
                    for c in range(2):
                        tpp = tsp.tile([P, 2, P], F16, name="tpp")
                        nc.tensor.transpose(
                            out=tpp[:, 0, :tn],
                            in_=est[:tn, c * P : (c + 1) * P],
                            identity=ident[:tn, :tn],
                        )
                        nc.vector.tensor_scalar_mul(
                            xa[:, c, to : to + tn], tpp[:, 0, :tn], SA
                        )
                    # 44 leftover dims as [22, 2]: (256+p, 278+p)
                    tpp2 = tsp.tile([P, 2, P], F16, name="tpp")
                    nc.tensor.transpose(
                        out=tpp2[:22, 0, :tn],
                        in_=est[:tn, 256:278],
                        identity=ident[:tn, :tn],
                    )
                    nc.tensor.transpose(
                        out=tpp2[:22, 1, :tn],
                        in_=est[:tn, 278:300],
                        identity=ident[:tn, :tn],
                    )
                    nc.vector.tensor_scalar_mul(
                        xb[:22, :, to : to + tn], tpp2[:22, :, :tn], SA
                    )
                return (xa, xb)

            def l1_layer(x_cur, p_, t):
                """Layer 1: fp8 DR matmuls + wide-2 activations (bias is
                folded into the xb pack). Returns 4 wide gate tiles
                [128, 4, PW] in order i, f, g, o (f None at t0)."""
                t0 = t == 0
                xa, xb = x_cur
                ga = [None] * 4
                for gt in range(4):          # gate type: i, f, g, o
                    if t0 and gt == 1:
                        continue
                    gw = gb.tile([P, 4 * PW], F16, name="gt")
                    for half in range(2):    # chunk pair (4gt+2*half, +1)
                        ps = gp.tile([P, 2, 512], F32, name="ps")
                        for jj in range(2):
                            mi = 4 * gt + 2 * half + jj
                            dst = ps[:, jj, :PW]
                            nc.tensor.matmul(
                                dst,
                                lhsT=q11d[:, :, mi * P : (mi + 1) * P],
                                rhs=xa[:],
                                start=True,
                                stop=False,
                                perf_mode=DRM,
                            )
                            nc.tensor.matmul(
                                dst,
                                lhsT=q11r[:, :, mi * P : (mi + 1) * P],
                                rhs=xb[:],
                                start=False,
                                stop=t0,
                                perf_mode=DRM,
                            )
                            if not t0:
                                for kb in range(2):
                                    nc.tensor.matmul(
                                        dst,
                                        lhsT=q1h[kb][:, :, mi * P : (mi + 1) * P],
                                        rhs=h1d[(t - 1) % 2][p_][:, 2 * kb * PW : (2 * kb + 2) * PW].rearrange("p (j n) -> p j n", j=2),
                                        start=False,
                                        stop=kb == 1,
                                        perf_mode=DRM,
                                    )
                        func = AFT.Tanh if gt == 2 else AFT.Sigmoid
                        nc.scalar.activation(
                            out=gw[:, 2 * half * PW : (2 * half + 2) * PW],
                            in_=ps[:, :, :PW],
                            func=func,
                            scale=DESC,
                        )
                    ga[gt] = gw
                return ga

            def l2_layer(p_, t):
                """Layer 2: fp8 DR (t<K8) or bf16 (t>=K8) matmuls + narrow
                activations carrying the b2 bias."""
                t0 = t == 0
                fp8 = t < K8
                ga = [None] * 4
                for gt in range(4):
                    if t0 and gt == 1:
                        continue
                    gw = gb.tile([P, 4 * PW], F16, name="gt")
                    for half in range(2):
                        ps = gp.tile([P, 2, 512], F32, name="ps")
                        for jj in range(2):
                            mi = 4 * gt + 2 * half + jj
                            dst = ps[:, jj, :PW]
                            first = True
                            if not t0:
                                if fp8:
                                    for kb in range(2):
                                        nc.tensor.matmul(
                                            dst,
                                            lhsT=q2h[kb][:, :, mi * P : (mi + 1) * P],
                                            rhs=h2d[(t - 1) % 2][p_][:, 2 * kb * PW : (2 * kb + 2) * PW].rearrange("p (j n) -> p j n", j=2),
                                            start=first,
                                            stop=False,
                                            perf_mode=DRM,
                                        )
                                        first = False
                                else:
                                    for k in range(4):
                                        nc.tensor.matmul(
                                            dst,
                                            lhsT=w2hb[k][:, mi * P : (mi + 1) * P],
                                            rhs=h2b[p_][:, k * PW : (k + 1) * PW],
                                            start=first,
                                            stop=False,
                                        )
                                        first = False
                            if fp8:
                                for kb in range(2):
                                    nc.tensor.matmul(
                                        dst,
                                        lhsT=q2i[kb][:, :, mi * P : (mi + 1) * P],
                                        rhs=h1d[t % 2][p_][:, 2 * kb * PW : (2 * kb + 2) * PW].rearrange("p (j n) -> p j n", j=2),
                                        start=first,
                                        stop=kb == 1,
                                        perf_mode=DRM,
                                    )
                                    first = False
                            else:
                                h1w = h1b_cur[0]
                                for k in range(4):
                                    nc.tensor.matmul(
                                        dst,
                                        lhsT=w2ib[k][:, mi * P : (mi + 1) * P],
                                        rhs=h1w[:, k * PW : (k + 1) * PW],
                                        start=first,
                                        stop=k == 3,
                                    )
                                    first = False
                        func = AFT.Tanh if gt == 2 else AFT.Sigmoid
                        for jj in range(2):
                            mi = 4 * gt + 2 * half + jj
                            nc.scalar.activation(
                                out=gw[:, (2 * half + jj) * PW : (2 * half + jj + 1) * PW],
                                in_=ps[:, jj, :PW],
                                func=func,
                                bias=b2_sb[:, mi : mi + 1],
                                scale=DESC,
                            )
                    ga[gt] = gw
                return ga

            def update(ga, c, t0, out8=None, outb=None, outf=None):
                """Wide c update + h writes. ga = [i, f, g, o] wide tiles.
                out8: fp8 DR tile [128,4,PW] (written as SA*h); outb: bf16
                wide tile; outf: fp32 wide tile."""
                gi, gf, gg, go = ga
                if t0:
                    nc.vector.tensor_mul(out=c[:], in0=gi[:], in1=gg[:])
                else:
                    p1 = tp.tile([P, 4 * PW], F16, name="tpt")
                    nc.vector.tensor_mul(out=p1[:], in0=gf[:], in1=c[:])
                    p2 = tp.tile([P, 4 * PW], F16, name="tpt")
                    nc.vector.tensor_mul(out=p2[:], in0=gi[:], in1=gg[:])
                    nc.vector.tensor_add(out=c[:], in0=p1[:], in1=p2[:])
                th = tp.tile([P, 4 * PW], F16, name="tpt")
                nc.scalar.activation(out=th[:], in_=c[:], func=AFT.Tanh)
                ndst = (out8 is not None) + (outb is not None) + (outf is not None)
                if ndst > 1:
                    hf = tp.tile([P, 4 * PW], F16, name="tpt")
                    nc.vector.tensor_mul(out=hf[:], in0=go[:], in1=th[:])
                    if out8 is not None:
                        nc.vector.tensor_scalar_mul(out8[:], hf[:], SA)
                    if outb is not None:
                        nc.vector.tensor_copy(out=outb[:], in_=hf[:])
                    if outf is not None:
                        nc.vector.tensor_copy(out=outf[:], in_=hf[:])
                elif out8 is not None:
                    # (go * SA) * th -> fp8 in one fused op
                    nc.vector.scalar_tensor_tensor(
                        out=out8[:], in0=go[:], scalar=SA, in1=th[:],
                        op0=mybir.AluOpType.mult, op1=mybir.AluOpType.mult,
                    )
                elif outb is not None:
                    nc.vector.tensor_mul(out=outb[:], in0=go[:], in1=th[:])
                else:
                    nc.vector.tensor_mul(out=outf[:], in0=go[:], in1=th[:])

            NS = T * NPASS
            ests = {0: gen_gather(0, 0), 1: gen_gather(1, 1)}
            x_cur = gen_x(ests.pop(0), 0)
            h1b_cur = [None]
            for t in range(T):
                for p_ in range(NPASS):
                    s = t * NPASS + p_
                    wb = t % 2
                    t0 = t == 0
                    tail = t >= K8
                    g1 = l1_layer(x_cur, p_, t)
                    # h1 destinations: fp8 for next-step L1 and fp8-L2;
                    # bf16 for the bf16 L2 path
                    o8 = h1d[wb][p_] if (t < T - 1 or not tail) else None
                    if tail:
                        h1b_cur[0] = hb.tile([P, 4 * PW], BF16, name="h1b")
                    update(g1, c1[p_], t0, out8=o8,
                           outb=h1b_cur[0] if tail else None)
                    # prefetch: gathers two slots ahead, transposes one
                    # slot ahead (fills the PE gap before L2's h1 wait)
                    if s + 2 < NS:
                        ests[s + 2] = gen_gather(s + 2, (s + 2) % 2)
                    if s + 1 < NS:
                        x_next = gen_x(ests.pop(s + 1), (s + 1) % 2)
                    else:
                        x_next = None
                    g2 = l2_layer(p_, t)
                    o8 = h2d[wb][p_] if t + 1 < K8 else None
                    ob = h2b[p_] if K8 - 1 <= t < T - 1 else None
                    if t == T - 1:
                        of = hb.tile([P, 4 * PW], F32, name="h2f")
                    else:
                        of = None
                    update(g2, c2[p_], t0, out8=o8, outb=ob, outf=of)
                    if t == T - 1:
                        for r in range(4):
                            nc.sync.dma_start(
                                out=outd[r * P : (r + 1) * P, p_ * PW : (p_ + 1) * PW],
                                in_=of[:, r * PW : (r + 1) * PW],
                            )
                    x_cur = x_next
    _split_waits(nc)
    return nc


_NC_CACHE = None


def _get_nc():
    global _NC_CACHE
    if _NC_CACHE is None:
        _NC_CACHE = build_nc()
    return _NC_CACHE


def _sigmoid(x):
    return 1.0 / (1.0 + np.exp(-x))


def _gptq_e4m3(W, X, sc):
    """GPTQ-quantize W [M,K] to e4m3 at scale sc, calibrated on inputs
    X [N,K]. Returns the scaled quantized weights (fp32 values of sc*W)."""
    K = W.shape[1]
    H = (X.T @ X) / max(len(X), 1)
    H[np.diag_indices(K)] += 0.01 * np.mean(np.diag(H)) + 1e-8
    Hinv = np.linalg.inv(H)
    Wq = (W * sc).astype(np.float32).copy()
    Q = np.zeros_like(Wq)
    for j in range(K):
        q = Wq[:, j].astype(NP8).astype(np.float32)
        Q[:, j] = q
        err = (Wq[:, j] - q) / Hinv[j, j]
        if j + 1 < K:
            Wq[:, j + 1 :] -= np.outer(err, Hinv[j, j + 1 :])
    return Q


def _prep_core_inputs(sentence, word2vec, W_ih1, W_hh1, b_ih1, b_hh1,
                      W_ih2, W_hh2, b_ih2, b_hh2):
    f = lambda a: np.ascontiguousarray(np.asarray(a), dtype=np.float32)
    ids_all = np.asarray(sentence).reshape(BN, T).astype(np.int32)
    w2v = f(word2vec)
    w2v16 = np.tanh(w2v).astype(np.float16)
    W0 = {"W_ih1": f(W_ih1), "W_hh1": f(W_hh1),
          "W_ih2": f(W_ih2), "W_hh2": f(W_hh2)}
    b1 = f(b_ih1) + f(b_hh1)
    b2 = f(b_ih2) + f(b_hh2)

    # ---- bf16 weights (layer-2 late steps), pre-scaled x512 so the
    # PSUM scale matches the fp8 DR path ----
    fw = lambda a: np.ascontiguousarray((a.T * (SW * SA)).astype(NPBF))
    w2ib = fw(W0["W_ih2"])
    w2hb = fw(W0["W_hh2"])

    # ---- calibration run (CPU, fp32) for GPTQ Hessians ----
    CAL = 512
    cs = ids_all[:: max(BN // CAL, 1)][:CAL]
    h1 = np.zeros((CAL, RNN), np.float32)
    c1 = np.zeros_like(h1)
    h2 = np.zeros_like(h1)
    c2 = np.zeros_like(h1)
    Xx, Xh1, Xh1b, Xh2 = [], [], [], []
    for t in range(T):
        x = w2v16[cs[:, t]].astype(np.float32)
        Xx.append(x)
        Xh1.append(h1.copy())
        Xh2.append(h2.copy())
        g1 = x @ W0["W_ih1"].T + h1 @ W0["W_hh1"].T + b1
        i_, f_, g_, o_ = np.split(g1, 4, axis=1)
        c1 = _sigmoid(f_) * c1 + _sigmoid(i_) * np.tanh(g_)
        h1 = _sigmoid(o_) * np.tanh(c1)
        Xh1b.append(h1.copy())
        g2 = h1 @ W0["W_ih2"].T + h2 @ W0["W_hh2"].T + b2
        i_, f_, g_, o_ = np.split(g2, 4, axis=1)
        c2 = _sigmoid(f_) * c2 + _sigmoid(i_) * np.tanh(g_)
        h2 = _sigmoid(o_) * np.tanh(c2)

    Q = {}
    Q["W_ih1"] = _gptq_e4m3(W0["W_ih1"], np.concatenate(Xx) * SA, SW)
    Q["W_hh1"] = _gptq_e4m3(W0["W_hh1"], np.concatenate(Xh1) * SA, SW)
    Q["W_ih2"] = _gptq_e4m3(W0["W_ih2"], np.concatenate(Xh1b[:K8]) * SA, SW)
    Q["W_hh2"] = _gptq_e4m3(W0["W_hh2"], np.concatenate(Xh2[:K8]) * SA, SW)

    # ---- DR packs: tile[ki, j, m] = Q[m, 256*kb + 128*j + ki] ----
    def dr_pack(Qm, kb):
        lo = Qm[:, 256 * kb : 256 * kb + 128].T          # [128, M]
        hi = Qm[:, 256 * kb + 128 : 256 * kb + 256].T    # [128, M]
        return np.ascontiguousarray(
            np.stack([lo, hi], axis=1).astype(NP8))       # [128, 2, M]

    q11d = dr_pack(Q["W_ih1"], 0)
    # xb pack [23, 2, M]: rows p<22 = emb dims (256+p, 278+p); row 22 is
    # the bias (rhs value SA): j=0 main e4m3(64*b1), j=1 fp8 residual
    q11r = np.zeros((P, 2, 4 * RNN), np.float32)
    q11r[:22, 0, :] = Q["W_ih1"][:, 256:278].T
    q11r[:22, 1, :] = Q["W_ih1"][:, 278:300].T
    bmain = (SW * b1).astype(NP8).astype(np.float32)
    bres = (SW * b1 - bmain).astype(NP8).astype(np.float32)
    q11r[22, 0, :] = bmain
    q11r[22, 1, :] = bres
    q11r = np.ascontiguousarray(q11r.astype(NP8))
    q1h = np.stack([dr_pack(Q["W_hh1"], kb) for kb in range(2)])
    q2i = np.stack([dr_pack(Q["W_ih2"], kb) for kb in range(2)])
    q2h = np.stack([dr_pack(Q["W_hh2"], kb) for kb in range(2)])

    b2m = f(b2.reshape(NM, P).T)

    in_maps = []
    for k in range(NCORES):
        ids_k = ids_all[k * BNC : (k + 1) * BNC]
        ids_arr = np.zeros((P, P), dtype=np.int32)
        for t in range(T):
            for p_ in range(NPASS):
                s = t * NPASS + p_
                for j, (to, tn) in enumerate(TOKT):
                    g = s * len(TOKT) + j
                    ids_arr[:tn, g] = ids_k[p_ * PW + to : p_ * PW + to + tn, t]
        xbc = np.zeros((P, 2, PW), np.float32)
        xbc[22, :, :] = SA
        in_maps.append(
            {
                "xbc": xbc.astype(NP8),
                "w2v16": w2v16,
                "q11d": q11d,
                "q11r": q11r,
                "q1h": q1h,
                "q2i": q2i,
                "q2h": q2h,
                "w2ib": w2ib,
                "w2hb": w2hb,
                "b2": b2m,
                "ids": ids_arr,
            }
        )
    return in_maps


def kernel(sentence, word2vec, W_ih1, W_hh1, b_ih1, b_hh1,
           W_ih2, W_hh2, b_ih2, b_hh2, _trace=False, _return_perf=None):
    nc = _get_nc()
    in_maps = _prep_core_inputs(
        sentence, word2vec, W_ih1, W_hh1, b_ih1, b_hh1, W_ih2, W_hh2, b_ih2, b_hh2
    )
    res = run_bass_kernel_spmd(
        nc, in_maps, core_ids=list(range(NCORES)), trace=_trace
    )
    if _return_perf is not None:
        _return_perf.append(res)
    parts = [res.results[k]["out"].T for k in range(NCORES)]
    out = np.concatenate(parts, axis=0).reshape(B, NCLS, RNN)
    return np.ascontiguousarray(out, dtype=np.float32)


# revision 17
# speedup vs baseline: 1.2387x; 1.0467x over previous
"""Trainium2 Bass kernel for nn_ClassEmbedding: embedding gather + tanh
feeding a 2-layer LSTM (hidden 512, T=8) over a fused batch of 12800,
data-parallel over 8 NeuronCores (1600 rows/core).

V3: step-major pass interleaving. The 1600 columns per core split into 4
passes of 400; the loop is for t: for p, so the 4 independent recurrence
chains hide each other's gate->h->gate dependency latency.

Layout: everything transposed. Gates are computed as
    gatesT[4R, B] = W_ihT-contract(xT) + W_hhT-contract(hT)
so hidden states live as hT/cT [512 -> 4x128 chunks, B] and the recurrence
needs zero transposes. Only the 300-dim embeddings are transposed (PE
transpose, 128-token tiles).

Precision: L1 runs fp8 DoubleRow at every step (its error is damped by
layer-2's gate slope). L2 runs fp8 DR for t < K8 and bf16 (weights
pre-scaled x512 so the PSUM scale matches the DR path) for t >= K8.
All K-parts of L1's x-contribution are DR: xa covers emb dims 0..255,
xb [23,2] covers dims 256..299 plus a bias row (value SA in the rhs,
64*b1 + fp8-residual in the lhsT) so b1 lands in PSUM and the layer-1
activations can run wide over chunk pairs with a plain scale=1/512.
Weights for the fp8 path are GPTQ-quantized on the host against
calibration activations from a small CPU reference run.
"""
import sys

sys.path.insert(0, "/opt/trn_rl_repo")

import numpy as np
import ml_dtypes

from concourse import bass, mybir
import concourse.tile as tile
from concourse.bass_utils import run_bass_kernel_spmd
from concourse.masks import make_identity
from concourse.vector_clock import ScopedClock

F32 = mybir.dt.float32
F16 = mybir.dt.float16
BF16 = mybir.dt.bfloat16
F8 = mybir.dt.float8e4
I32 = mybir.dt.int32
AFT = mybir.ActivationFunctionType
DRM = mybir.MatmulPerfMode.DoubleRow
NP8 = ml_dtypes.float8_e4m3
NPBF = ml_dtypes.bfloat16

P = 128
VOCAB, EMB, RNN, T = 20000, 300, 512, 8
B, NCLS = 64, 200
BN = B * NCLS            # 12800
NCORES = 8
BNC = BN // NCORES       # 1600 per core
PW = 400                 # pass width (batch columns per pass)
NPASS = BNC // PW        # 4
NM = 16                  # 2048 / 128 gate row chunks
TOKT = [(0, 128), (128, 128), (256, 128), (384, 16)]  # token tiles per pass

K8 = 6                   # steps t < K8 run layer-2 in fp8; the rest bf16
SW = 64.0                # fp8 weight scale
SA = 8.0                 # fp8 activation scale (products scaled SW*SA = 512)
DESC = 1.0 / (SW * SA)


def _patched_drain_and_barrier(self, tick_clock, wait_clock):
    # walrus rejects >2 sync waits on one instruction; spread the final
    # drain's waits across single-wait NOPs.
    nc = self.nc
    drain_inst = nc.sync.drain()
    wait_clock.add_sem_waits(
        drain_inst.ins, ScopedClock({None: tick_clock.global_clock})
    )
    si = drain_inst.ins.sync_info
    if si is not None and si.on_wait and len(si.on_wait) > 1:
        waits = list(si.on_wait)
        si.on_wait = waits[:1]
        for w in waits[1:]:
            nop = nc.sync.nop()
            nop.ins.sync_info = mybir.SyncInfo(on_wait=[w], on_update=[])
    nc.all_engine_barrier()
    assert self.sems is not None
    popped = nc._tile_sem_poison_stack.pop()
    assert popped is self._sem_poison
    nc.clear_and_free_semaphores(list(self.sems.allocated().values()))
    nc.all_engine_barrier()


tile.TileContext._drain_and_barrier = _patched_drain_and_barrier


def _split_waits(nc, maxw=1):
    """walrus rejects instructions carrying more than a couple of sync
    waits; keep at most `maxw` on each instruction and move the rest to
    preceding same-engine NOPs."""
    wid = 0
    for bb in nc.main_func.blocks:
        out = []
        changed = False
        for inst in bb.instructions:
            si = inst.sync_info
            if si is not None and si.on_wait and len(si.on_wait) > maxw:
                waits = list(si.on_wait)
                for w in waits[maxw:]:
                    nop = mybir.InstNoOp(name=f"wsplit-{wid}", ins=[], outs=[])
                    wid += 1
                    nop.engine = inst.engine
                    nop.sync_info = mybir.SyncInfo(on_wait=[w], on_update=[])
                    out.append(nop)
                inst.sync_info = mybir.SyncInfo(
                    on_wait=waits[:maxw], on_update=list(si.on_update or [])
                )
                changed = True
            out.append(inst)
        if changed:
            bb.instructions = out


def build_nc():
    nc = bass.Bass()
    w2v16 = nc.declare_dram_parameter("w2v16", [VOCAB, EMB], F16, isOutput=False)
    q11d_d = nc.declare_dram_parameter("q11d", [P, 2, 4 * RNN], F8, isOutput=False)
    q11r_d = nc.declare_dram_parameter("q11r", [P, 2, 4 * RNN], F8, isOutput=False)
    q1h_d = nc.declare_dram_parameter("q1h", [2, P, 2, 4 * RNN], F8, isOutput=False)
    q2i_d = nc.declare_dram_parameter("q2i", [2, P, 2, 4 * RNN], F8, isOutput=False)
    q2h_d = nc.declare_dram_parameter("q2h", [2, P, 2, 4 * RNN], F8, isOutput=False)
    w2ib_d = nc.declare_dram_parameter("w2ib", [RNN, 4 * RNN], BF16, isOutput=False)
    w2hb_d = nc.declare_dram_parameter("w2hb", [RNN, 4 * RNN], BF16, isOutput=False)
    b2d = nc.declare_dram_parameter("b2", [P, NM], F32, isOutput=False)
    idsd = nc.declare_dram_parameter("ids", [P, P], I32, isOutput=False)
    xbc_d = nc.declare_dram_parameter("xbc", [P, 2, PW], F8, isOutput=False)
    outd = nc.declare_dram_parameter("out", [RNN, BNC], F32, isOutput=True)

    with tile.TileContext(nc) as tc:
        with (
            tc.tile_pool(name="wp", bufs=1) as wp,
            tc.tile_pool(name="sp", bufs=1) as sp,
            tc.tile_pool(name="gb", bufs=10) as gb,
            tc.tile_pool(name="tp", bufs=6) as tp,
            tc.tile_pool(name="hb", bufs=2) as hb,
            tc.tile_pool(name="gp", bufs=3, space="PSUM") as gp,
            tc.tile_pool(name="tsp", bufs=2, space="PSUM") as tsp,
        ):
            # ---- small constants first: the sync DMA queue is FIFO, and
            # the gather pipeline only needs ids ----
            ids_sb = wp.tile([P, P], I32, name="ids_sb")
            nc.sync.dma_start(out=ids_sb[:], in_=idsd[:])
            b2_sb = wp.tile([P, NM], F32, name="b2_sb")
            nc.sync.dma_start(out=b2_sb[:], in_=b2d[:])
            ident32 = wp.tile([P, P], F32, name="ident32")
            make_identity(nc, ident32[:])
            ident = wp.tile([P, P], F16, name="ident")
            nc.vector.tensor_copy(out=ident[:], in_=ident32[:])

            # persistent embedding staging ring: 8 tiles = 2 slots of
            # gather prefetch so PE transposes never wait on the gather
            est_ring = [wp.tile([P, EMB], F16, name=f"est{i}") for i in range(8)]

            # ---- fp8 weights (needed first: step 0 is fp8) ----
            q11d = wp.tile([P, 2, 4 * RNN], F8, name="q11d")
            nc.sync.dma_start(out=q11d[:], in_=q11d_d[:])
            q11r = wp.tile([P, 2, 4 * RNN], F8, name="q11r")
            nc.sync.dma_start(out=q11r[:], in_=q11r_d[:])

            def load_dr_w(dram, label):
                chunks = []
                for i in range(2):
                    wt = wp.tile([P, 2, 4 * RNN], F8, name=f"{label}_{i}")
                    nc.sync.dma_start(out=wt[:], in_=dram[i])
                    chunks.append(wt)
                return chunks

            q1h = load_dr_w(q1h_d, "q1h")
            q2i = load_dr_w(q2i_d, "q2i")
            q2h = load_dr_w(q2h_d, "q2h")

            # ---- bf16 weights (layer-2 late steps), pre-scaled x512 ----
            def load_bf_w(dram, label):
                chunks = []
                for i in range(4):
                    wt = wp.tile([P, 4 * RNN], BF16, name=f"{label}_{i}")
                    nc.sync.dma_start(out=wt[:], in_=dram[i * P : (i + 1) * P, :])
                    chunks.append(wt)
                return chunks

            w2ib = load_bf_w(w2ib_d, "w2ib")
            w2hb = load_bf_w(w2hb_d, "w2hb")

            # ---- persistent state tiles (per pass) ----
            # fp8 DR layout: wide [128, 4, PW]; r-chunk r = (kb=r//2, j=r%2)
            # so the DR rhs for kb is tile[:, 2kb:2kb+2, :]
            h1d = [
                [sp.tile([P, 4 * PW], F8, name=f"h1d_{bb}_{p_}") for p_ in range(NPASS)]
                for bb in range(2)
            ]
            h2d = [
                [sp.tile([P, 4 * PW], F8, name=f"h2d_{bb}_{p_}") for p_ in range(NPASS)]
                for bb in range(2)
            ]
            # bf16 h2 for the tail steps (h2(t) read at t+1 >= K8)
            h2b = [sp.tile([P, 4 * PW], BF16, name=f"h2b_{p_}") for p_ in range(NPASS)]
            c1 = [sp.tile([P, 4 * PW], F16, name=f"c1_{p_}") for p_ in range(NPASS)]
            c2 = [sp.tile([P, 4 * PW], F16, name=f"c2_{p_}") for p_ in range(NPASS)]
            # x double buffers (persistent so the xb bias row survives)
            xa_db = [sp.tile([P, 2, PW], F8, name=f"xa{i}") for i in range(2)]
            xb_db = [sp.tile([P, 2, PW], F8, name=f"xb{i}") for i in range(2)]
            for i in range(2):
                nc.sync.dma_start(out=xb_db[i][:], in_=xbc_d[:])


            def gen_gather(s, slot):
                """Issue the 4 indirect gathers for slot s into est ring
                half `slot` (0/1)."""
                ests = []
                for j, (to, tn) in enumerate(TOKT):
                    g = s * len(TOKT) + j
                    est = est_ring[slot * 4 + j]
                    nc.gpsimd.indirect_dma_start(
                        out=est[:tn, :],
                        out_offset=None,
                        in_=w2v16[:],
                        in_offset=bass.IndirectOffsetOnAxis(
                            ap=ids_sb[:tn, g : g + 1], axis=0
                        ),
                    )
                    ests.append(est)
                return ests

            def gen_x(ests, slot):
                """PE-transpose gathered [tokens, kchunk] tiles into the
                fp8 DR rhs tiles (xa: emb 0..255, xb: 256..299), applying
                the x8 activation scale during the PSUM->SBUF descale copy.
                Copies alternate vector/gpsimd to balance engines."""
                xa = xa_db[slot]
                xb = xb_db[slot]
                nv = 0
                for j, (to, tn) in enumerate(TOKT):
                    est = ests[j]
                    tpp = tsp.tile([P, 2, P], F16, name="tpp")
                    for c in range(2):
                        nc.tensor.transpose(
                            out=tpp[:, c, :tn],
                            in_=est[:tn, c * P : (c + 1) * P],
                            identity=ident[:tn, :tn],
                        )
                    nc.vector.tensor_scalar_mul(
                        xa[:, :, to : to + tn], tpp[:, :, :tn], SA
                    )
                    # 44 leftover dims as [22, 2]: (256+p, 278+p)
                    tpp2 = tsp.tile([P, 2, P], F16, name="tpp")
                    nc.tensor.transpose(
                        out=tpp2[:22, 0, :tn],
                        in_=est[:tn, 256:278],
                        identity=ident[:tn, :tn],
                    )
                    nc.tensor.transpose(
                        out=tpp2[:22, 1, :tn],
                        in_=est[:tn, 278:300],
                        identity=ident[:tn, :tn],
                    )
                    nc.vector.tensor_scalar_mul(
                        xb[:22, :, to : to + tn], tpp2[:22, :, :tn], SA
                    )
                return (xa, xb)

            def l1_layer(x_cur, p_, t):
                """Layer 1: fp8 DR matmuls + wide-2 activations (bias is
                folded into the xb pack). Returns 4 wide gate tiles
                [128, 4, PW] in order i, f, g, o (f None at t0)."""
                t0 = t == 0
                xa, xb = x_cur
                ga = [None] * 4
                for gt in range(4):          # gate type: i, f, g, o
                    if t0 and gt == 1:
                        continue
                    gw = gb.tile([P, 4 * PW], F16, name="gt")
                    for half in range(2):    # chunk pair (4gt+2*half, +1)
                        ps = gp.tile([P, 2, 512], F32, name="ps")
                        for jj in range(2):
                            mi = 4 * gt + 2 * half + jj
                            dst = ps[:, jj, :PW]
                            nc.tensor.matmul(
                                dst,
                                lhsT=q11d[:, :, mi * P : (mi + 1) * P],
                                rhs=xa[:],
                                start=True,
                                stop=False,
                                perf_mode=DRM,
                            )
                            nc.tensor.matmul(
                                dst,
                                lhsT=q11r[:, :, mi * P : (mi + 1) * P],
                                rhs=xb[:],
                                start=False,
                                stop=t0,
                                perf_mode=DRM,
                            )
                            if not t0:
                                for kb in range(2):
                                    nc.tensor.matmul(
                                        dst,
                                        lhsT=q1h[kb][:, :, mi * P : (mi + 1) * P],
                                        rhs=h1d[(t - 1) % 2][p_][:, 2 * kb * PW : (2 * kb + 2) * PW].rearrange("p (j n) -> p j n", j=2),
                                        start=False,
                                        stop=kb == 1,
                                        perf_mode=DRM,
                                    )
                        func = AFT.Tanh if gt == 2 else AFT.Sigmoid
                        nc.scalar.activation(
                            out=gw[:, 2 * half * PW : (2 * half + 2) * PW],
                            in_=ps[:, :, :PW],
                            func=func,
                            scale=DESC,
                        )
                    ga[gt] = gw
                return ga

            def l2_layer(p_, t):
                """Layer 2: fp8 DR (t<K8) or bf16 (t>=K8) matmuls + narrow
                activations carrying the b2 bias."""
                t0 = t == 0
                fp8 = t < K8
                ga = [None] * 4
                for gt in range(4):
                    if t0 and gt == 1:
                        continue
                    gw = gb.tile([P, 4 * PW], F16, name="gt")
                    for half in range(2):
                        ps = gp.tile([P, 2, 512], F32, name="ps")
                        for jj in range(2):
                            mi = 4 * gt + 2 * half + jj
                            dst = ps[:, jj, :PW]
                            first = True
                            if not t0:
                                if fp8:
                                    for kb in range(2):
                                        nc.tensor.matmul(
                                            dst,
                                            lhsT=q2h[kb][:, :, mi * P : (mi + 1) * P],
                                            rhs=h2d[(t - 1) % 2][p_][:, 2 * kb * PW : (2 * kb + 2) * PW].rearrange("p (j n) -> p j n", j=2),
                                            start=first,
                                            stop=False,
                                            perf_mode=DRM,
                                        )
                                        first = False
                                else:
                                    for k in range(4):
                                        nc.tensor.matmul(
                                            dst,
                                            lhsT=w2hb[k][:, mi * P : (mi + 1) * P],
                                            rhs=h2b[p_][:, k * PW : (k + 1) * PW],
                                            start=first,
                                            stop=False,
                                        )
                                        first = False
                            if fp8:
                                for kb in range(2):
                                    nc.tensor.matmul(
                                        dst,
                                        lhsT=q2i[kb][:, :, mi * P : (mi + 1) * P],
                                        rhs=h1d[t % 2][p_][:, 2 * kb * PW : (2 * kb + 2) * PW].rearrange("p (j n) -> p j n", j=2),
                                        start=first,
                                        stop=kb == 1,
                                        perf_mode=DRM,
                                    )
                                    first = False
                            else:
                                h1w = h1b_cur[0]
                                for k in range(4):
                                    nc.tensor.matmul(
                                        dst,
                                        lhsT=w2ib[k][:, mi * P : (mi + 1) * P],
                                        rhs=h1w[:, k * PW : (k + 1) * PW],
                                        start=first,
                                        stop=k == 3,
                                    )
                                    first = False
                        func = AFT.Tanh if gt == 2 else AFT.Sigmoid
                        for jj in range(2):
                            mi = 4 * gt + 2 * half + jj
                            nc.scalar.activation(
                                out=gw[:, (2 * half + jj) * PW : (2 * half + jj + 1) * PW],
                                in_=ps[:, jj, :PW],
                                func=func,
                                bias=b2_sb[:, mi : mi + 1],
                                scale=DESC,
                            )
                    ga[gt] = gw
                return ga

            def update(ga, c, t0, out8=None, outb=None, outf=None,
                       halves=False):
                """c update + h writes over [128, 4*PW] tiles. ga = [i, f,
                g, o]. out8: fp8 DR tile (written as SA*h); outb: bf16;
                outf: fp32. halves=True splits into kb-halves so the first
                half's h lands ~2.5us earlier (emission order avoids DVE
                FIFO head-blocking)."""
                gi, gf, gg, go = ga
                HW = 2 * PW
                cuts = [(0, HW), (HW, HW)] if halves else [(0, 4 * PW)]
                ths = []
                for o, w in cuts:
                    sl = slice(o, o + w)
                    if t0:
                        nc.vector.tensor_mul(out=c[:, sl], in0=gi[:, sl],
                                             in1=gg[:, sl])
                    else:
                        p1 = tp.tile([P, 4 * PW], F16, name="tpt")
                        nc.vector.tensor_mul(out=p1[:, sl], in0=gf[:, sl],
                                             in1=c[:, sl])
                        p2 = tp.tile([P, 4 * PW], F16, name="tpt")
                        nc.vector.tensor_mul(out=p2[:, sl], in0=gi[:, sl],
                                             in1=gg[:, sl])
                        nc.vector.tensor_add(out=c[:, sl], in0=p1[:, sl],
                                             in1=p2[:, sl])
                    th = tp.tile([P, 4 * PW], F16, name="tpt")
                    nc.scalar.activation(out=th[:, sl], in_=c[:, sl],
                                         func=AFT.Tanh)
                    ths.append(th)
                ndst = (out8 is not None) + (outb is not None) + (outf is not None)
                for (o, w), th in zip(cuts, ths):
                    sl = slice(o, o + w)
                    if ndst > 1:
                        hf = tp.tile([P, 4 * PW], F16, name="tpt")
                        nc.vector.tensor_mul(out=hf[:, sl], in0=go[:, sl],
                                             in1=th[:, sl])
                        if out8 is not None:
                            nc.vector.tensor_scalar_mul(out8[:, sl],
                                                        hf[:, sl], SA)
                        if outb is not None:
                            nc.vector.tensor_copy(out=outb[:, sl],
                                                  in_=hf[:, sl])
                        if outf is not None:
                            nc.vector.tensor_copy(out=outf[:, sl],
                                                  in_=hf[:, sl])
                    elif out8 is not None:
                        # (go * SA) * th -> fp8 in one fused op
                        nc.vector.scalar_tensor_tensor(
                            out=out8[:, sl], in0=go[:, sl], scalar=SA,
                            in1=th[:, sl],
                            op0=mybir.AluOpType.mult, op1=mybir.AluOpType.mult,
                        )
                    elif outb is not None:
                        nc.vector.tensor_mul(out=outb[:, sl], in0=go[:, sl],
                                             in1=th[:, sl])
                    else:
                        nc.vector.tensor_mul(out=outf[:, sl], in0=go[:, sl],
                                             in1=th[:, sl])

            NS = T * NPASS
            ests = {0: gen_gather(0, 0), 1: gen_gather(1, 1)}
            x_cur = gen_x(ests.pop(0), 0)
            h1b_cur = [None]
            for t in range(T):
                for p_ in range(NPASS):
                    s = t * NPASS + p_
                    wb = t % 2
                    t0 = t == 0
                    tail = t >= K8
                    g1 = l1_layer(x_cur, p_, t)
                    # h1 destinations: fp8 for next-step L1 and fp8-L2;
                    # bf16 for the bf16 L2 path
                    o8 = h1d[wb][p_] if (t < T - 1 or not tail) else None
                    if tail:
                        h1b_cur[0] = hb.tile([P, 4 * PW], BF16, name="h1b")
                    update(g1, c1[p_], t0, out8=o8,
                           outb=h1b_cur[0] if tail else None, halves=True)
                    # prefetch: gathers two slots ahead, transposes one
                    # slot ahead (fills the PE gap before L2's h1 wait)
                    if s + 2 < NS:
                        ests[s + 2] = gen_gather(s + 2, (s + 2) % 2)
                    if s + 1 < NS:
                        x_next = gen_x(ests.pop(s + 1), (s + 1) % 2)
                    else:
                        x_next = None
                    g2 = l2_layer(p_, t)
                    o8 = h2d[wb][p_] if t + 1 < K8 else None
                    ob = h2b[p_] if K8 - 1 <= t < T - 1 else None
                    if t == T - 1:
                        of = hb.tile([P, 4 * PW], F32, name="h2f")
                    else:
                        of = None
                    update(g2, c2[p_], t0, out8=o8, outb=ob, outf=of)
                    if t == T - 1:
                        for r in range(4):
                            nc.sync.dma_start(
                                out=outd[r * P : (r + 1) * P, p_ * PW : (p_ + 1) * PW],
                                in_=of[:, r * PW : (r + 1) * PW],
                            )
                    x_cur = x_next
    _split_waits(nc)
    return nc


_NC_CACHE = None


def _get_nc():
    global _NC_CACHE
    if _NC_CACHE is None:
        _NC_CACHE = build_nc()
    return _NC_CACHE


def _sigmoid(x):
    return 1.0 / (1.0 + np.exp(-x))


def _gptq_e4m3(W, X, sc):
    """GPTQ-quantize W [M,K] to e4m3 at scale sc, calibrated on inputs
    X [N,K]. Returns the scaled quantized weights (fp32 values of sc*W)."""
    K = W.shape[1]
    H = (X.T @ X) / max(len(X), 1)
    H[np.diag_indices(K)] += 0.01 * np.mean(np.diag(H)) + 1e-8
    Hinv = np.linalg.inv(H)
    Wq = (W * sc).astype(np.float32).copy()
    Q = np.zeros_like(Wq)
    for j in range(K):
        q = Wq[:, j].astype(NP8).astype(np.float32)
        Q[:, j] = q
        err = (Wq[:, j] - q) / Hinv[j, j]
        if j + 1 < K:
            Wq[:, j + 1 :] -= np.outer(err, Hinv[j, j + 1 :])
    return Q


def _prep_core_inputs(sentence, word2vec, W_ih1, W_hh1, b_ih1, b_hh1,
                      W_ih2, W_hh2, b_ih2, b_hh2):
    f = lambda a: np.ascontiguousarray(np.asarray(a), dtype=np.float32)
    ids_all = np.asarray(sentence).reshape(BN, T).astype(np.int32)
    w2v = f(word2vec)
    w2v16 = np.tanh(w2v).astype(np.float16)
    W0 = {"W_ih1": f(W_ih1), "W_hh1": f(W_hh1),
          "W_ih2": f(W_ih2), "W_hh2": f(W_hh2)}
    b1 = f(b_ih1) + f(b_hh1)
    b2 = f(b_ih2) + f(b_hh2)

    # ---- bf16 weights (layer-2 late steps), pre-scaled x512 so the
    # PSUM scale matches the fp8 DR path ----
    fw = lambda a: np.ascontiguousarray((a.T * (SW * SA)).astype(NPBF))
    w2ib = fw(W0["W_ih2"])
    w2hb = fw(W0["W_hh2"])

    # ---- calibration run (CPU, fp32) for GPTQ Hessians ----
    CAL = 512
    cs = ids_all[:: max(BN // CAL, 1)][:CAL]
    h1 = np.zeros((CAL, RNN), np.float32)
    c1 = np.zeros_like(h1)
    h2 = np.zeros_like(h1)
    c2 = np.zeros_like(h1)
    Xx, Xh1, Xh1b, Xh2 = [], [], [], []
    for t in range(T):
        x = w2v16[cs[:, t]].astype(np.float32)
        Xx.append(x)
        Xh1.append(h1.copy())
        Xh2.append(h2.copy())
        g1 = x @ W0["W_ih1"].T + h1 @ W0["W_hh1"].T + b1
        i_, f_, g_, o_ = np.split(g1, 4, axis=1)
        c1 = _sigmoid(f_) * c1 + _sigmoid(i_) * np.tanh(g_)
        h1 = _sigmoid(o_) * np.tanh(c1)
        Xh1b.append(h1.copy())
        g2 = h1 @ W0["W_ih2"].T + h2 @ W0["W_hh2"].T + b2
        i_, f_, g_, o_ = np.split(g2, 4, axis=1)
        c2 = _sigmoid(f_) * c2 + _sigmoid(i_) * np.tanh(g_)
        h2 = _sigmoid(o_) * np.tanh(c2)

    Q = {}
    Q["W_ih1"] = _gptq_e4m3(W0["W_ih1"], np.concatenate(Xx) * SA, SW)
    Q["W_hh1"] = _gptq_e4m3(W0["W_hh1"], np.concatenate(Xh1) * SA, SW)
    Q["W_ih2"] = _gptq_e4m3(W0["W_ih2"], np.concatenate(Xh1b[:K8]) * SA, SW)
    Q["W_hh2"] = _gptq_e4m3(W0["W_hh2"], np.concatenate(Xh2[:K8]) * SA, SW)

    # ---- DR packs: tile[ki, j, m] = Q[m, 256*kb + 128*j + ki] ----
    def dr_pack(Qm, kb):
        lo = Qm[:, 256 * kb : 256 * kb + 128].T          # [128, M]
        hi = Qm[:, 256 * kb + 128 : 256 * kb + 256].T    # [128, M]
        return np.ascontiguousarray(
            np.stack([lo, hi], axis=1).astype(NP8))       # [128, 2, M]

    q11d = dr_pack(Q["W_ih1"], 0)
    # xb pack [23, 2, M]: rows p<22 = emb dims (256+p, 278+p); row 22 is
    # the bias (rhs value SA): j=0 main e4m3(64*b1), j=1 fp8 residual
    q11r = np.zeros((P, 2, 4 * RNN), np.float32)
    q11r[:22, 0, :] = Q["W_ih1"][:, 256:278].T
    q11r[:22, 1, :] = Q["W_ih1"][:, 278:300].T
    bmain = (SW * b1).astype(NP8).astype(np.float32)
    bres = (SW * b1 - bmain).astype(NP8).astype(np.float32)
    q11r[22, 0, :] = bmain
    q11r[22, 1, :] = bres
    q11r = np.ascontiguousarray(q11r.astype(NP8))
    q1h = np.stack([dr_pack(Q["W_hh1"], kb) for kb in range(2)])
    q2i = np.stack([dr_pack(Q["W_ih2"], kb) for kb in range(2)])
    q2h = np.stack([dr_pack(Q["W_hh2"], kb) for kb in range(2)])

    b2m = f(b2.reshape(NM, P).T)

    in_maps = []
    for k in range(NCORES):
        ids_k = ids_all[k * BNC : (k + 1) * BNC]
        ids_arr = np.zeros((P, P), dtype=np.int32)
        for t in range(T):
            for p_ in range(NPASS):
                s = t * NPASS + p_
                for j, (to, tn) in enumerate(TOKT):
                    g = s * len(TOKT) + j
                    ids_arr[:tn, g] = ids_k[p_ * PW + to : p_ * PW + to + tn, t]
        xbc = np.zeros((P, 2, PW), np.float32)
        xbc[22, :, :] = SA
        in_maps.append(
            {
                "xbc": xbc.astype(NP8),
                "w2v16": w2v16,
                "q11d": q11d,
                "q11r": q11r,
                "q1h": q1h,
                "q2i": q2i,
                "q2h": q2h,
                "w2ib": w2ib,
                "w2hb": w2hb,
                "b2": b2m,
                "ids": ids_arr,
            }
        )
    return in_maps


def kernel(sentence, word2vec, W_ih1, W_hh1, b_ih1, b_hh1,
           W_ih2, W_hh2, b_ih2, b_hh2, _trace=False, _return_perf=None):
    nc = _get_nc()
    in_maps = _prep_core_inputs(
        sentence, word2vec, W_ih1, W_hh1, b_ih1, b_hh1, W_ih2, W_hh2, b_ih2, b_hh2
    )
    res = run_bass_kernel_spmd(
        nc, in_maps, core_ids=list(range(NCORES)), trace=_trace
    )
    if _return_perf is not None:
        _return_perf.append(res)
    parts = [res.results[k]["out"].T for k in range(NCORES)]
    out = np.concatenate(parts, axis=0).reshape(B, NCLS, RNN)
    return np.ascontiguousarray(out, dtype=np.float32)


# revision 20
# speedup vs baseline: 1.2435x; 1.0039x over previous
"""Trainium2 Bass kernel for nn_ClassEmbedding: embedding gather + tanh
feeding a 2-layer LSTM (hidden 512, T=8) over a fused batch of 12800,
data-parallel over 8 NeuronCores (1600 rows/core).

V3: step-major pass interleaving. The 1600 columns per core split into 4
passes of 400; the loop is for t: for p, so the 4 independent recurrence
chains hide each other's gate->h->gate dependency latency.

Layout: everything transposed. Gates are computed as
    gatesT[4R, B] = W_ihT-contract(xT) + W_hhT-contract(hT)
so hidden states live as hT/cT [512 -> 4x128 chunks, B] and the recurrence
needs zero transposes. Only the 300-dim embeddings are transposed (PE
transpose, 128-token tiles).

Precision: L1 runs fp8 DoubleRow at every step (its error is damped by
layer-2's gate slope). L2 runs fp8 DR for t < K8 and bf16 (weights
pre-scaled x512 so the PSUM scale matches the DR path) for t >= K8.
All K-parts of L1's x-contribution are DR: xa covers emb dims 0..255,
xb [23,2] covers dims 256..299 plus a bias row (value SA in the rhs,
64*b1 + fp8-residual in the lhsT) so b1 lands in PSUM and the layer-1
activations can run wide over chunk pairs with a plain scale=1/512.
Weights for the fp8 path are GPTQ-quantized on the host against
calibration activations from a small CPU reference run.
"""
import sys

sys.path.insert(0, "/opt/trn_rl_repo")

import numpy as np
import ml_dtypes

from concourse import bass, mybir
import concourse.tile as tile
from concourse.bass_utils import run_bass_kernel_spmd
from concourse.masks import make_identity
from concourse.vector_clock import ScopedClock

F32 = mybir.dt.float32
F16 = mybir.dt.float16
BF16 = mybir.dt.bfloat16
F8 = mybir.dt.float8e4
I32 = mybir.dt.int32
AFT = mybir.ActivationFunctionType
DRM = mybir.MatmulPerfMode.DoubleRow
NP8 = ml_dtypes.float8_e4m3
NPBF = ml_dtypes.bfloat16

P = 128
VOCAB, EMB, RNN, T = 20000, 300, 512, 8
B, NCLS = 64, 200
BN = B * NCLS            # 12800
NCORES = 8
BNC = BN // NCORES       # 1600 per core
PW = 400                 # pass width (batch columns per pass)
NPASS = BNC // PW        # 4
NM = 16                  # 2048 / 128 gate row chunks
TOKT = [(0, 128), (128, 128), (256, 128), (384, 16)]  # token tiles per pass

K8 = 6                   # steps t < K8 run layer-2 in fp8; the rest bf16
SW = 64.0                # fp8 weight scale
SA = 8.0                 # fp8 activation scale (products scaled SW*SA = 512)
DESC = 1.0 / (SW * SA)


def _patched_drain_and_barrier(self, tick_clock, wait_clock):
    # walrus rejects >2 sync waits on one instruction; spread the final
    # drain's waits across single-wait NOPs.
    nc = self.nc
    drain_inst = nc.sync.drain()
    wait_clock.add_sem_waits(
        drain_inst.ins, ScopedClock({None: tick_clock.global_clock})
    )
    si = drain_inst.ins.sync_info
    if si is not None and si.on_wait and len(si.on_wait) > 1:
        waits = list(si.on_wait)
        si.on_wait = waits[:1]
        for w in waits[1:]:
            nop = nc.sync.nop()
            nop.ins.sync_info = mybir.SyncInfo(on_wait=[w], on_update=[])
    nc.all_engine_barrier()
    assert self.sems is not None
    popped = nc._tile_sem_poison_stack.pop()
    assert popped is self._sem_poison
    nc.clear_and_free_semaphores(list(self.sems.allocated().values()))
    nc.all_engine_barrier()


tile.TileContext._drain_and_barrier = _patched_drain_and_barrier


def _split_waits(nc, maxw=1):
    """walrus rejects instructions carrying more than a couple of sync
    waits; keep at most `maxw` on each instruction and move the rest to
    preceding same-engine NOPs."""
    wid = 0
    for bb in nc.main_func.blocks:
        out = []
        changed = False
        for inst in bb.instructions:
            si = inst.sync_info
            if si is not None and si.on_wait and len(si.on_wait) > maxw:
                waits = list(si.on_wait)
                for w in waits[maxw:]:
                    nop = mybir.InstNoOp(name=f"wsplit-{wid}", ins=[], outs=[])
                    wid += 1
                    nop.engine = inst.engine
                    nop.sync_info = mybir.SyncInfo(on_wait=[w], on_update=[])
                    out.append(nop)
                inst.sync_info = mybir.SyncInfo(
                    on_wait=waits[:maxw], on_update=list(si.on_update or [])
                )
                changed = True
            out.append(inst)
        if changed:
            bb.instructions = out


def build_nc():
    nc = bass.Bass()
    w2v16 = nc.declare_dram_parameter("w2v16", [VOCAB, EMB], F16, isOutput=False)
    q11d_d = nc.declare_dram_parameter("q11d", [P, 2, 4 * RNN], F8, isOutput=False)
    q11r_d = nc.declare_dram_parameter("q11r", [P, 2, 4 * RNN], F8, isOutput=False)
    q1h_d = nc.declare_dram_parameter("q1h", [2, P, 2, 4 * RNN], F8, isOutput=False)
    q2i_d = nc.declare_dram_parameter("q2i", [2, P, 2, 4 * RNN], F8, isOutput=False)
    q2h_d = nc.declare_dram_parameter("q2h", [2, P, 2, 4 * RNN], F8, isOutput=False)
    w2ib_d = nc.declare_dram_parameter("w2ib", [RNN, 4 * RNN], BF16, isOutput=False)
    w2hb_d = nc.declare_dram_parameter("w2hb", [RNN, 4 * RNN], BF16, isOutput=False)
    b2d = nc.declare_dram_parameter("b2", [P, NM], F32, isOutput=False)
    idsd = nc.declare_dram_parameter("ids", [P, P], I32, isOutput=False)
    xbc_d = nc.declare_dram_parameter("xbc", [P, 2, PW], F8, isOutput=False)
    outd = nc.declare_dram_parameter("out", [RNN, BNC], F32, isOutput=True)

    with tile.TileContext(nc) as tc:
        with (
            tc.tile_pool(name="wp", bufs=1) as wp,
            tc.tile_pool(name="sp", bufs=1) as sp,
            tc.tile_pool(name="gb", bufs=12) as gb,
            tc.tile_pool(name="tp", bufs=8) as tp,
            tc.tile_pool(name="hb", bufs=1) as hb,
            tc.tile_pool(name="gp", bufs=3, space="PSUM") as gp,
            tc.tile_pool(name="tsp", bufs=2, space="PSUM") as tsp,
        ):
            # ---- small constants first: the sync DMA queue is FIFO, and
            # the gather pipeline only needs ids ----
            ids_sb = wp.tile([P, P], I32, name="ids_sb")
            nc.sync.dma_start(out=ids_sb[:], in_=idsd[:])
            b2_sb = wp.tile([P, NM], F32, name="b2_sb")
            nc.sync.dma_start(out=b2_sb[:], in_=b2d[:])
            ident32 = wp.tile([P, P], F32, name="ident32")
            make_identity(nc, ident32[:])
            ident = wp.tile([P, P], F16, name="ident")
            nc.vector.tensor_copy(out=ident[:], in_=ident32[:])

            # persistent embedding staging ring: 8 tiles = 2 slots of
            # gather prefetch so PE transposes never wait on the gather
            est_ring = [wp.tile([P, EMB], F16, name=f"est{i}") for i in range(8)]

            # ---- fp8 weights (needed first: step 0 is fp8) ----
            q11d = wp.tile([P, 2, 4 * RNN], F8, name="q11d")
            nc.sync.dma_start(out=q11d[:], in_=q11d_d[:])
            q11r = wp.tile([P, 2, 4 * RNN], F8, name="q11r")
            nc.sync.dma_start(out=q11r[:], in_=q11r_d[:])

            def load_dr_w(dram, label):
                chunks = []
                for i in range(2):
                    wt = wp.tile([P, 2, 4 * RNN], F8, name=f"{label}_{i}")
                    nc.sync.dma_start(out=wt[:], in_=dram[i])
                    chunks.append(wt)
                return chunks

            q1h = load_dr_w(q1h_d, "q1h")
            q2i = load_dr_w(q2i_d, "q2i")
            q2h = load_dr_w(q2h_d, "q2h")

            # ---- bf16 weights (layer-2 late steps), pre-scaled x512 ----
            def load_bf_w(dram, label):
                chunks = []
                for i in range(4):
                    wt = wp.tile([P, 4 * RNN], BF16, name=f"{label}_{i}")
                    nc.sync.dma_start(out=wt[:], in_=dram[i * P : (i + 1) * P, :])
                    chunks.append(wt)
                return chunks

            w2ib = load_bf_w(w2ib_d, "w2ib")
            w2hb = load_bf_w(w2hb_d, "w2hb")

            # ---- persistent state tiles (per pass) ----
            # fp8 DR layout: wide [128, 4, PW]; r-chunk r = (kb=r//2, j=r%2)
            # so the DR rhs for kb is tile[:, 2kb:2kb+2, :]
            h1d = [
                [sp.tile([P, 4 * PW], F8, name=f"h1d_{bb}_{p_}") for p_ in range(NPASS)]
                for bb in range(2)
            ]
            h2d = [
                [sp.tile([P, 4 * PW], F8, name=f"h2d_{bb}_{p_}") for p_ in range(NPASS)]
                for bb in range(2)
            ]
            # bf16 h2 for the tail steps (h2(t) read at t+1 >= K8)
            h2b = [sp.tile([P, 4 * PW], BF16, name=f"h2b_{p_}") for p_ in range(NPASS)]
            c1 = [sp.tile([P, 4 * PW], F16, name=f"c1_{p_}") for p_ in range(NPASS)]
            c2 = [sp.tile([P, 4 * PW], F16, name=f"c2_{p_}") for p_ in range(NPASS)]
            # x double buffers (persistent so the xb bias row survives)
            xa_db = [sp.tile([P, 2, PW], F8, name=f"xa{i}") for i in range(2)]
            xb_db = [sp.tile([P, 2, PW], F8, name=f"xb{i}") for i in range(2)]
            for i in range(2):
                nc.sync.dma_start(out=xb_db[i][:], in_=xbc_d[:])


            def gen_gather(s, slot):
                """Issue the 4 indirect gathers for slot s into est ring
                half `slot` (0/1)."""
                ests = []
                for j, (to, tn) in enumerate(TOKT):
                    g = s * len(TOKT) + j
                    est = est_ring[slot * 4 + j]
                    nc.gpsimd.indirect_dma_start(
                        out=est[:tn, :],
                        out_offset=None,
                        in_=w2v16[:],
                        in_offset=bass.IndirectOffsetOnAxis(
                            ap=ids_sb[:tn, g : g + 1], axis=0
                        ),
                    )
                    ests.append(est)
                return ests

            def gen_x(ests, slot):
                """PE-transpose gathered [tokens, kchunk] tiles into the
                fp8 DR rhs tiles (xa: emb 0..255, xb: 256..299), applying
                the x8 activation scale during the PSUM->SBUF descale copy.
                Copies alternate vector/gpsimd to balance engines."""
                xa = xa_db[slot]
                xb = xb_db[slot]
                nv = 0
                for j, (to, tn) in enumerate(TOKT):
                    est = ests[j]
                    tpp = tsp.tile([P, 2, P], F16, name="tpp")
                    for c in range(2):
                        nc.tensor.transpose(
                            out=tpp[:, c, :tn],
                            in_=est[:tn, c * P : (c + 1) * P],
                            identity=ident[:tn, :tn],
                        )
                    nc.vector.tensor_scalar_mul(
                        xa[:, :, to : to + tn], tpp[:, :, :tn], SA
                    )
                    # 44 leftover dims as [22, 2]: (256+p, 278+p)
                    tpp2 = tsp.tile([P, 2, P], F16, name="tpp")
                    nc.tensor.transpose(
                        out=tpp2[:22, 0, :tn],
                        in_=est[:tn, 256:278],
                        identity=ident[:tn, :tn],
                    )
                    nc.tensor.transpose(
                        out=tpp2[:22, 1, :tn],
                        in_=est[:tn, 278:300],
                        identity=ident[:tn, :tn],
                    )
                    nc.vector.tensor_scalar_mul(
                        xb[:22, :, to : to + tn], tpp2[:22, :, :tn], SA
                    )
                return (xa, xb)

            def l1_layer(x_cur, p_, t):
                """Layer 1: fp8 DR matmuls + wide-2 activations (bias is
                folded into the xb pack). Returns 4 wide gate tiles
                [128, 4, PW] in order i, f, g, o (f None at t0)."""
                t0 = t == 0
                xa, xb = x_cur
                ga = [None] * 4
                for gt in range(4):          # gate type: i, f, g, o
                    if t0 and gt == 1:
                        continue
                    gw = gb.tile([P, 4 * PW], F16, name="gt")
                    for half in range(2):    # chunk pair (4gt+2*half, +1)
                        ps = gp.tile([P, 2, 512], F32, name="ps")
                        for jj in range(2):
                            mi = 4 * gt + 2 * half + jj
                            dst = ps[:, jj, :PW]
                            nc.tensor.matmul(
                                dst,
                                lhsT=q11d[:, :, mi * P : (mi + 1) * P],
                                rhs=xa[:],
                                start=True,
                                stop=False,
                                perf_mode=DRM,
                            )
                            nc.tensor.matmul(
                                dst,
                                lhsT=q11r[:, :, mi * P : (mi + 1) * P],
                                rhs=xb[:],
                                start=False,
                                stop=t0,
                                perf_mode=DRM,
                            )
                            if not t0:
                                for kb in range(2):
                                    nc.tensor.matmul(
                                        dst,
                                        lhsT=q1h[kb][:, :, mi * P : (mi + 1) * P],
                                        rhs=h1d[(t - 1) % 2][p_][:, 2 * kb * PW : (2 * kb + 2) * PW].rearrange("p (j n) -> p j n", j=2),
                                        start=False,
                                        stop=kb == 1,
                                        perf_mode=DRM,
                                    )
                        func = AFT.Tanh if gt == 2 else AFT.Sigmoid
                        nc.scalar.activation(
                            out=gw[:, 2 * half * PW : (2 * half + 2) * PW],
                            in_=ps[:, :, :PW],
                            func=func,
                            scale=DESC,
                        )
                    ga[gt] = gw
                return ga

            def l2_layer(p_, t):
                """Layer 2: fp8 DR (t<K8) or bf16 (t>=K8) matmuls + narrow
                activations carrying the b2 bias."""
                t0 = t == 0
                fp8 = t < K8
                ga = [None] * 4
                for gt in range(4):
                    if t0 and gt == 1:
                        continue
                    gw = gb.tile([P, 4 * PW], F16, name="gt")
                    for half in range(2):
                        ps = gp.tile([P, 2, 512], F32, name="ps")
                        for jj in range(2):
                            mi = 4 * gt + 2 * half + jj
                            dst = ps[:, jj, :PW]
                            first = True
                            if not t0:
                                if fp8:
                                    for kb in range(2):
                                        nc.tensor.matmul(
                                            dst,
                                            lhsT=q2h[kb][:, :, mi * P : (mi + 1) * P],
                                            rhs=h2d[(t - 1) % 2][p_][:, 2 * kb * PW : (2 * kb + 2) * PW].rearrange("p (j n) -> p j n", j=2),
                                            start=first,
                                            stop=False,
                                            perf_mode=DRM,
                                        )
                                        first = False
                                else:
                                    for k in range(4):
                                        nc.tensor.matmul(
                                            dst,
                                            lhsT=w2hb[k][:, mi * P : (mi + 1) * P],
                                            rhs=h2b[p_][:, k * PW : (k + 1) * PW],
                                            start=first,
                                            stop=False,
                                        )
                                        first = False
                            if fp8:
                                for kb in range(2):
                                    nc.tensor.matmul(
                                        dst,
                                        lhsT=q2i[kb][:, :, mi * P : (mi + 1) * P],
                                        rhs=h1d[t % 2][p_][:, 2 * kb * PW : (2 * kb + 2) * PW].rearrange("p (j n) -> p j n", j=2),
                                        start=first,
                                        stop=kb == 1,
                                        perf_mode=DRM,
                                    )
                                    first = False
                            else:
                                h1w = h1b_cur[0]
                                for k in range(4):
                                    nc.tensor.matmul(
                                        dst,
                                        lhsT=w2ib[k][:, mi * P : (mi + 1) * P],
                                        rhs=h1w[:, k * PW : (k + 1) * PW],
                                        start=first,
                                        stop=k == 3,
                                    )
                                    first = False
                        func = AFT.Tanh if gt == 2 else AFT.Sigmoid
                        for jj in range(2):
                            mi = 4 * gt + 2 * half + jj
                            nc.scalar.activation(
                                out=gw[:, (2 * half + jj) * PW : (2 * half + jj + 1) * PW],
                                in_=ps[:, jj, :PW],
                                func=func,
                                bias=b2_sb[:, mi : mi + 1],
                                scale=DESC,
                            )
                    ga[gt] = gw
                return ga

            def update(ga, c, t0, out8=None, outb=None, outf=None,
                       halves=False):
                """c update + h writes over [128, 4*PW] tiles. ga = [i, f,
                g, o]. out8: fp8 DR tile (written as SA*h); outb: bf16;
                outf: fp32. halves=True splits into kb-halves so the first
                half's h lands ~2.5us earlier (emission order avoids DVE
                FIFO head-blocking)."""
                gi, gf, gg, go = ga
                HW = 2 * PW
                cuts = [(0, HW), (HW, HW)] if halves else [(0, 4 * PW)]
                ths = []
                for o, w in cuts:
                    sl = slice(o, o + w)
                    if t0:
                        nc.vector.tensor_mul(out=c[:, sl], in0=gi[:, sl],
                                             in1=gg[:, sl])
                    else:
                        p1 = tp.tile([P, 4 * PW], F16, name="tpt")
                        nc.vector.tensor_mul(out=p1[:, sl], in0=gf[:, sl],
                                             in1=c[:, sl])
                        p2 = tp.tile([P, 4 * PW], F16, name="tpt")
                        nc.vector.tensor_mul(out=p2[:, sl], in0=gi[:, sl],
                                             in1=gg[:, sl])
                        nc.vector.tensor_add(out=c[:, sl], in0=p1[:, sl],
                                             in1=p2[:, sl])
                    th = tp.tile([P, 4 * PW], F16, name="tpt")
                    nc.scalar.activation(out=th[:, sl], in_=c[:, sl],
                                         func=AFT.Tanh)
                    ths.append(th)
                ndst = (out8 is not None) + (outb is not None) + (outf is not None)
                for (o, w), th in zip(cuts, ths):
                    sl = slice(o, o + w)
                    if ndst > 1:
                        hf = tp.tile([P, 4 * PW], F16, name="tpt")
                        nc.vector.tensor_mul(out=hf[:, sl], in0=go[:, sl],
                                             in1=th[:, sl])
                        if out8 is not None:
                            nc.vector.tensor_scalar_mul(out8[:, sl],
                                                        hf[:, sl], SA)
                        if outb is not None:
                            nc.vector.tensor_copy(out=outb[:, sl],
                                                  in_=hf[:, sl])
                        if outf is not None:
                            nc.vector.tensor_copy(out=outf[:, sl],
                                                  in_=hf[:, sl])
                    elif out8 is not None:
                        # (go * SA) * th -> fp8 in one fused op
                        nc.vector.scalar_tensor_tensor(
                            out=out8[:, sl], in0=go[:, sl], scalar=SA,
                            in1=th[:, sl],
                            op0=mybir.AluOpType.mult, op1=mybir.AluOpType.mult,
                        )
                    elif outb is not None:
                        nc.vector.tensor_mul(out=outb[:, sl], in0=go[:, sl],
                                             in1=th[:, sl])
                    else:
                        nc.vector.tensor_mul(out=outf[:, sl], in0=go[:, sl],
                                             in1=th[:, sl])

            NS = T * NPASS
            ests = {0: gen_gather(0, 0), 1: gen_gather(1, 1)}
            x_cur = gen_x(ests.pop(0), 0)
            h1b_cur = [None]
            for t in range(T):
                for p_ in range(NPASS):
                    s = t * NPASS + p_
                    wb = t % 2
                    t0 = t == 0
                    tail = t >= K8
                    g1 = l1_layer(x_cur, p_, t)
                    # h1 destinations: fp8 for next-step L1 and fp8-L2;
                    # bf16 for the bf16 L2 path
                    o8 = h1d[wb][p_] if (t < T - 1 or not tail) else None
                    if tail:
                        h1b_cur[0] = hb.tile([P, 4 * PW], BF16, name="h1b")
                    update(g1, c1[p_], t0, out8=o8,
                           outb=h1b_cur[0] if tail else None, halves=True)
                    # prefetch: gathers two slots ahead, transposes one
                    # slot ahead (fills the PE gap before L2's h1 wait)
                    if s + 2 < NS:
                        ests[s + 2] = gen_gather(s + 2, (s + 2) % 2)
                    if s + 1 < NS:
                        x_next = gen_x(ests.pop(s + 1), (s + 1) % 2)
                    else:
                        x_next = None
                    g2 = l2_layer(p_, t)
                    o8 = h2d[wb][p_] if t + 1 < K8 else None
                    ob = h2b[p_] if K8 - 1 <= t < T - 1 else None
                    if t == T - 1:
                        of = hb.tile([P, 4 * PW], F32, name="h2f")
                    else:
                        of = None
                    update(g2, c2[p_], t0, out8=o8, outb=ob, outf=of, halves=True)
                    if t == T - 1:
                        for r in range(4):
                            nc.sync.dma_start(
                                out=outd[r * P : (r + 1) * P, p_ * PW : (p_ + 1) * PW],
                                in_=of[:, r * PW : (r + 1) * PW],
                            )
                    x_cur = x_next
    _split_waits(nc)
    return nc


_NC_CACHE = None


def _get_nc():
    global _NC_CACHE
    if _NC_CACHE is None:
        _NC_CACHE = build_nc()
    return _NC_CACHE


def _sigmoid(x):
    return 1.0 / (1.0 + np.exp(-x))


def _gptq_e4m3(W, X, sc):
    """GPTQ-quantize W [M,K] to e4m3 at scale sc, calibrated on inputs
    X [N,K]. Returns the scaled quantized weights (fp32 values of sc*W)."""
    K = W.shape[1]
    H = (X.T @ X) / max(len(X), 1)
    H[np.diag_indices(K)] += 0.01 * np.mean(np.diag(H)) + 1e-8
    Hinv = np.linalg.inv(H)
    Wq = (W * sc).astype(np.float32).copy()
    Q = np.zeros_like(Wq)
    for j in range(K):
        q = Wq[:, j].astype(NP8).astype(np.float32)
        Q[:, j] = q
        err = (Wq[:, j] - q) / Hinv[j, j]
        if j + 1 < K:
            Wq[:, j + 1 :] -= np.outer(err, Hinv[j, j + 1 :])
    return Q


def _prep_core_inputs(sentence, word2vec, W_ih1, W_hh1, b_ih1, b_hh1,
                      W_ih2, W_hh2, b_ih2, b_hh2):
    f = lambda a: np.ascontiguousarray(np.asarray(a), dtype=np.float32)
    ids_all = np.asarray(sentence).reshape(BN, T).astype(np.int32)
    w2v = f(word2vec)
    w2v16 = np.tanh(w2v).astype(np.float16)
    W0 = {"W_ih1": f(W_ih1), "W_hh1": f(W_hh1),
          "W_ih2": f(W_ih2), "W_hh2": f(W_hh2)}
    b1 = f(b_ih1) + f(b_hh1)
    b2 = f(b_ih2) + f(b_hh2)

    # ---- bf16 weights (layer-2 late steps), pre-scaled x512 so the
    # PSUM scale matches the fp8 DR path ----
    fw = lambda a: np.ascontiguousarray((a.T * (SW * SA)).astype(NPBF))
    w2ib = fw(W0["W_ih2"])
    w2hb = fw(W0["W_hh2"])

    # ---- calibration run (CPU, fp32) for GPTQ Hessians ----
    CAL = 512
    cs = ids_all[:: max(BN // CAL, 1)][:CAL]
    h1 = np.zeros((CAL, RNN), np.float32)
    c1 = np.zeros_like(h1)
    h2 = np.zeros_like(h1)
    c2 = np.zeros_like(h1)
    Xx, Xh1, Xh1b, Xh2 = [], [], [], []
    for t in range(T):
        x = w2v16[cs[:, t]].astype(np.float32)
        Xx.append(x)
        Xh1.append(h1.copy())
        Xh2.append(h2.copy())
        g1 = x @ W0["W_ih1"].T + h1 @ W0["W_hh1"].T + b1
        i_, f_, g_, o_ = np.split(g1, 4, axis=1)
        c1 = _sigmoid(f_) * c1 + _sigmoid(i_) * np.tanh(g_)
        h1 = _sigmoid(o_) * np.tanh(c1)
        Xh1b.append(h1.copy())
        g2 = h1 @ W0["W_ih2"].T + h2 @ W0["W_hh2"].T + b2
        i_, f_, g_, o_ = np.split(g2, 4, axis=1)
        c2 = _sigmoid(f_) * c2 + _sigmoid(i_) * np.tanh(g_)
        h2 = _sigmoid(o_) * np.tanh(c2)

    Q = {}
    Q["W_ih1"] = _gptq_e4m3(W0["W_ih1"], np.concatenate(Xx) * SA, SW)
    Q["W_hh1"] = _gptq_e4m3(W0["W_hh1"], np.concatenate(Xh1) * SA, SW)
    Q["W_ih2"] = _gptq_e4m3(W0["W_ih2"], np.concatenate(Xh1b[:K8]) * SA, SW)
    Q["W_hh2"] = _gptq_e4m3(W0["W_hh2"], np.concatenate(Xh2[:K8]) * SA, SW)

    # ---- DR packs: tile[ki, j, m] = Q[m, 256*kb + 128*j + ki] ----
    def dr_pack(Qm, kb):
        lo = Qm[:, 256 * kb : 256 * kb + 128].T          # [128, M]
        hi = Qm[:, 256 * kb + 128 : 256 * kb + 256].T    # [128, M]
        return np.ascontiguousarray(
            np.stack([lo, hi], axis=1).astype(NP8))       # [128, 2, M]

    q11d = dr_pack(Q["W_ih1"], 0)
    # xb pack [23, 2, M]: rows p<22 = emb dims (256+p, 278+p); row 22 is
    # the bias (rhs value SA): j=0 main e4m3(64*b1), j=1 fp8 residual
    q11r = np.zeros((P, 2, 4 * RNN), np.float32)
    q11r[:22, 0, :] = Q["W_ih1"][:, 256:278].T
    q11r[:22, 1, :] = Q["W_ih1"][:, 278:300].T
    bmain = (SW * b1).astype(NP8).astype(np.float32)
    bres = (SW * b1 - bmain).astype(NP8).astype(np.float32)
    q11r[22, 0, :] = bmain
    q11r[22, 1, :] = bres
    q11r = np.ascontiguousarray(q11r.astype(NP8))
    q1h = np.stack([dr_pack(Q["W_hh1"], kb) for kb in range(2)])
    q2i = np.stack([dr_pack(Q["W_ih2"], kb) for kb in range(2)])
    q2h = np.stack([dr_pack(Q["W_hh2"], kb) for kb in range(2)])

    b2m = f(b2.reshape(NM, P).T)

    in_maps = []
    for k in range(NCORES):
        ids_k = ids_all[k * BNC : (k + 1) * BNC]
        ids_arr = np.zeros((P, P), dtype=np.int32)
        for t in range(T):
            for p_ in range(NPASS):
                s = t * NPASS + p_
                for j, (to, tn) in enumerate(TOKT):
                    g = s * len(TOKT) + j
                    ids_arr[:tn, g] = ids_k[p_ * PW + to : p_ * PW + to + tn, t]
        xbc = np.zeros((P, 2, PW), np.float32)
        xbc[22, :, :] = SA
        in_maps.append(
            {
                "xbc": xbc.astype(NP8),
                "w2v16": w2v16,
                "q11d": q11d,
                "q11r": q11r,
                "q1h": q1h,
                "q2i": q2i,
                "q2h": q2h,
                "w2ib": w2ib,
                "w2hb": w2hb,
                "b2": b2m,
                "ids": ids_arr,
            }
        )
    return in_maps


def kernel(sentence, word2vec, W_ih1, W_hh1, b_ih1, b_hh1,
           W_ih2, W_hh2, b_ih2, b_hh2, _trace=False, _return_perf=None):
    nc = _get_nc()
    in_maps = _prep_core_inputs(
        sentence, word2vec, W_ih1, W_hh1, b_ih1, b_hh1, W_ih2, W_hh2, b_ih2, b_hh2
    )
    res = run_bass_kernel_spmd(
        nc, in_maps, core_ids=list(range(NCORES)), trace=_trace
    )
    if _return_perf is not None:
        _return_perf.append(res)
    parts = [res.results[k]["out"].T for k in range(NCORES)]
    out = np.concatenate(parts, axis=0).reshape(B, NCLS, RNN)
    return np.ascontiguousarray(out, dtype=np.float32)


# revision 21
# speedup vs baseline: 1.2937x; 1.0403x over previous
"""Trainium2 Bass kernel for nn_ClassEmbedding: embedding gather + tanh
feeding a 2-layer LSTM (hidden 512, T=8) over a fused batch of 12800,
data-parallel over 8 NeuronCores (1600 rows/core).

V3: step-major pass interleaving. The 1600 columns per core split into 4
passes of 400; the loop is for t: for p, so the 4 independent recurrence
chains hide each other's gate->h->gate dependency latency.

Layout: everything transposed. Gates are computed as
    gatesT[4R, B] = W_ihT-contract(xT) + W_hhT-contract(hT)
so hidden states live as hT/cT [512 -> 4x128 chunks, B] and the recurrence
needs zero transposes. Only the 300-dim embeddings are transposed (PE
transpose, 128-token tiles).

Precision: L1 runs fp8 DoubleRow at every step (its error is damped by
layer-2's gate slope). L2 runs fp8 DR for t < K8 and bf16 (weights
pre-scaled x512 so the PSUM scale matches the DR path) for t >= K8.
All K-parts of L1's x-contribution are DR: xa covers emb dims 0..255,
xb [23,2] covers dims 256..299 plus a bias row (value SA in the rhs,
64*b1 + fp8-residual in the lhsT) so b1 lands in PSUM and the layer-1
activations can run wide over chunk pairs with a plain scale=1/512.
Weights for the fp8 path are GPTQ-quantized on the host against
calibration activations from a small CPU reference run.
"""
import sys

sys.path.insert(0, "/opt/trn_rl_repo")

import numpy as np
import ml_dtypes

from concourse import bass, mybir
import concourse.tile as tile
from concourse.bass_utils import run_bass_kernel_spmd
from concourse.masks import make_identity
from concourse.vector_clock import ScopedClock

F32 = mybir.dt.float32
F16 = mybir.dt.float16
BF16 = mybir.dt.bfloat16
F8 = mybir.dt.float8e4
I32 = mybir.dt.int32
AFT = mybir.ActivationFunctionType
DRM = mybir.MatmulPerfMode.DoubleRow
NP8 = ml_dtypes.float8_e4m3
NPBF = ml_dtypes.bfloat16

P = 128
VOCAB, EMB, RNN, T = 20000, 300, 512, 8
B, NCLS = 64, 200
BN = B * NCLS            # 12800
NCORES = 8
BNC = BN // NCORES       # 1600 per core
PW = 400                 # pass width (batch columns per pass)
NPASS = BNC // PW        # 4
NM = 16                  # 2048 / 128 gate row chunks
TOKT = [(0, 128), (128, 128), (256, 128), (384, 16)]  # token tiles per pass

K8 = 6                   # steps t < K8 run layer-2 in fp8; the rest bf16
SW = 64.0                # fp8 weight scale
SA = 8.0                 # fp8 activation scale (products scaled SW*SA = 512)
DESC = 1.0 / (SW * SA)


def _patched_drain_and_barrier(self, tick_clock, wait_clock):
    # walrus rejects >2 sync waits on one instruction; spread the final
    # drain's waits across single-wait NOPs.
    nc = self.nc
    drain_inst = nc.sync.drain()
    wait_clock.add_sem_waits(
        drain_inst.ins, ScopedClock({None: tick_clock.global_clock})
    )
    si = drain_inst.ins.sync_info
    if si is not None and si.on_wait and len(si.on_wait) > 1:
        waits = list(si.on_wait)
        si.on_wait = waits[:1]
        for w in waits[1:]:
            nop = nc.sync.nop()
            nop.ins.sync_info = mybir.SyncInfo(on_wait=[w], on_update=[])
    nc.all_engine_barrier()
    assert self.sems is not None
    popped = nc._tile_sem_poison_stack.pop()
    assert popped is self._sem_poison
    nc.clear_and_free_semaphores(list(self.sems.allocated().values()))
    nc.all_engine_barrier()


tile.TileContext._drain_and_barrier = _patched_drain_and_barrier


def _split_waits(nc, maxw=1):
    """walrus rejects instructions carrying more than a couple of sync
    waits; keep at most `maxw` on each instruction and move the rest to
    preceding same-engine NOPs."""
    wid = 0
    for bb in nc.main_func.blocks:
        out = []
        changed = False
        for inst in bb.instructions:
            si = inst.sync_info
            if si is not None and si.on_wait and len(si.on_wait) > maxw:
                waits = list(si.on_wait)
                for w in waits[maxw:]:
                    nop = mybir.InstNoOp(name=f"wsplit-{wid}", ins=[], outs=[])
                    wid += 1
                    nop.engine = inst.engine
                    nop.sync_info = mybir.SyncInfo(on_wait=[w], on_update=[])
                    out.append(nop)
                inst.sync_info = mybir.SyncInfo(
                    on_wait=waits[:maxw], on_update=list(si.on_update or [])
                )
                changed = True
            out.append(inst)
        if changed:
            bb.instructions = out


def build_nc():
    nc = bass.Bass()
    w2v16 = nc.declare_dram_parameter("w2v16", [VOCAB, EMB], F16, isOutput=False)
    q11d_d = nc.declare_dram_parameter("q11d", [P, 2, 4 * RNN], F8, isOutput=False)
    q11r_d = nc.declare_dram_parameter("q11r", [P, 2, 4 * RNN], F8, isOutput=False)
    q1h_d = nc.declare_dram_parameter("q1h", [2, P, 2, 4 * RNN], F8, isOutput=False)
    q2i_d = nc.declare_dram_parameter("q2i", [2, P, 2, 4 * RNN], F8, isOutput=False)
    q2h_d = nc.declare_dram_parameter("q2h", [2, P, 2, 4 * RNN], F8, isOutput=False)
    w2ib_d = nc.declare_dram_parameter("w2ib", [RNN, 4 * RNN], BF16, isOutput=False)
    w2hb_d = nc.declare_dram_parameter("w2hb", [RNN, 4 * RNN], BF16, isOutput=False)
    b2d = nc.declare_dram_parameter("b2", [P, NM], F32, isOutput=False)
    idsd = nc.declare_dram_parameter("ids", [P, P], I32, isOutput=False)
    xbc_d = nc.declare_dram_parameter("xbc", [P, 2, PW], F8, isOutput=False)
    outd = nc.declare_dram_parameter("out", [RNN, BNC], F32, isOutput=True)

    with tile.TileContext(nc) as tc:
        with (
            tc.tile_pool(name="wp", bufs=1) as wp,
            tc.tile_pool(name="sp", bufs=1) as sp,
            tc.tile_pool(name="gb", bufs=12) as gb,
            tc.tile_pool(name="tp", bufs=8) as tp,
            tc.tile_pool(name="hb", bufs=1) as hb,
            tc.tile_pool(name="gp", bufs=3, space="PSUM") as gp,
            tc.tile_pool(name="tsp", bufs=2, space="PSUM") as tsp,
        ):
            # ---- small constants first: the sync DMA queue is FIFO, and
            # the gather pipeline only needs ids ----
            ids_sb = wp.tile([P, P], I32, name="ids_sb")
            nc.sync.dma_start(out=ids_sb[:], in_=idsd[:])
            b2_sb = wp.tile([P, NM], F32, name="b2_sb")
            nc.sync.dma_start(out=b2_sb[:], in_=b2d[:])
            ident32 = wp.tile([P, P], F32, name="ident32")
            make_identity(nc, ident32[:])
            ident = wp.tile([P, P], F16, name="ident")
            nc.vector.tensor_copy(out=ident[:], in_=ident32[:])

            # persistent embedding staging ring: 8 tiles = 2 slots of
            # gather prefetch so PE transposes never wait on the gather
            est_ring = [wp.tile([P, EMB], F16, name=f"est{i}") for i in range(8)]

            # ---- fp8 weights (needed first: step 0 is fp8) ----
            q11d = wp.tile([P, 2, 4 * RNN], F8, name="q11d")
            nc.sync.dma_start(out=q11d[:], in_=q11d_d[:])
            q11r = wp.tile([P, 2, 4 * RNN], F8, name="q11r")
            nc.sync.dma_start(out=q11r[:], in_=q11r_d[:])

            def load_dr_w(dram, label):
                chunks = []
                for i in range(2):
                    wt = wp.tile([P, 2, 4 * RNN], F8, name=f"{label}_{i}")
                    nc.sync.dma_start(out=wt[:], in_=dram[i])
                    chunks.append(wt)
                return chunks

            q1h = load_dr_w(q1h_d, "q1h")
            q2i = load_dr_w(q2i_d, "q2i")
            q2h = load_dr_w(q2h_d, "q2h")

            # ---- bf16 weights (layer-2 late steps), pre-scaled x512 ----
            def load_bf_w(dram, label):
                chunks = []
                for i in range(4):
                    wt = wp.tile([P, 4 * RNN], BF16, name=f"{label}_{i}")
                    nc.sync.dma_start(out=wt[:], in_=dram[i * P : (i + 1) * P, :])
                    chunks.append(wt)
                return chunks

            w2ib = load_bf_w(w2ib_d, "w2ib")
            w2hb = load_bf_w(w2hb_d, "w2hb")

            # ---- persistent state tiles (per pass) ----
            # fp8 DR layout: wide [128, 4, PW]; r-chunk r = (kb=r//2, j=r%2)
            # so the DR rhs for kb is tile[:, 2kb:2kb+2, :]
            h1d = [
                [sp.tile([P, 4 * PW], F8, name=f"h1d_{bb}_{p_}") for p_ in range(NPASS)]
                for bb in range(2)
            ]
            h2d = [
                [sp.tile([P, 4 * PW], F8, name=f"h2d_{bb}_{p_}") for p_ in range(NPASS)]
                for bb in range(2)
            ]
            # bf16 h2 for the tail steps (h2(t) read at t+1 >= K8)
            h2b = [sp.tile([P, 4 * PW], BF16, name=f"h2b_{p_}") for p_ in range(NPASS)]
            c1 = [sp.tile([P, 4 * PW], F16, name=f"c1_{p_}") for p_ in range(NPASS)]
            c2 = [sp.tile([P, 4 * PW], F16, name=f"c2_{p_}") for p_ in range(NPASS)]
            # x double buffers (persistent so the xb bias row survives)
            xa_db = [sp.tile([P, 2, PW], F8, name=f"xa{i}") for i in range(2)]
            xb_db = [sp.tile([P, 2, PW], F8, name=f"xb{i}") for i in range(2)]
            for i in range(2):
                nc.sync.dma_start(out=xb_db[i][:], in_=xbc_d[:])


            def gen_gather(s, slot):
                """Issue the 4 indirect gathers for slot s into est ring
                half `slot` (0/1)."""
                ests = []
                for j, (to, tn) in enumerate(TOKT):
                    g = s * len(TOKT) + j
                    est = est_ring[slot * 4 + j]
                    nc.gpsimd.indirect_dma_start(
                        out=est[:tn, :],
                        out_offset=None,
                        in_=w2v16[:],
                        in_offset=bass.IndirectOffsetOnAxis(
                            ap=ids_sb[:tn, g : g + 1], axis=0
                        ),
                    )
                    ests.append(est)
                return ests

            def gen_x(ests, slot):
                """PE-transpose gathered [tokens, kchunk] tiles into the
                fp8 DR rhs tiles (xa: emb 0..255, xb: 256..299), applying
                the x8 activation scale during the PSUM->SBUF descale copy.
                Copies alternate vector/gpsimd to balance engines."""
                xa = xa_db[slot]
                xb = xb_db[slot]
                nv = 0
                for j, (to, tn) in enumerate(TOKT):
                    est = ests[j]
                    tpp = tsp.tile([P, 2, P], F16, name="tpp")
                    for c in range(2):
                        nc.tensor.transpose(
                            out=tpp[:, c, :tn],
                            in_=est[:tn, c * P : (c + 1) * P],
                            identity=ident[:tn, :tn],
                        )
                    nc.vector.tensor_scalar_mul(
                        xa[:, :, to : to + tn], tpp[:, :, :tn], SA
                    )
                    # 44 leftover dims as [22, 2]: (256+p, 278+p)
                    tpp2 = tsp.tile([P, 2, P], F16, name="tpp")
                    nc.tensor.transpose(
                        out=tpp2[:22, 0, :tn],
                        in_=est[:tn, 256:278],
                        identity=ident[:tn, :tn],
                    )
                    nc.tensor.transpose(
                        out=tpp2[:22, 1, :tn],
                        in_=est[:tn, 278:300],
                        identity=ident[:tn, :tn],
                    )
                    nc.vector.tensor_scalar_mul(
                        xb[:22, :, to : to + tn], tpp2[:22, :, :tn], SA
                    )
                return (xa, xb)

            def l1_layer(x_cur, p_, t):
                """Layer 1: fp8 DR matmuls + wide-2 activations (bias is
                folded into the xb pack). Returns 4 wide gate tiles
                [128, 4, PW] in order i, f, g, o (f None at t0)."""
                t0 = t == 0
                xa, xb = x_cur
                ga = [None] * 4
                for gt in range(4):          # gate type: i, f, g, o
                    if t0 and gt == 1:
                        continue
                    gw = gb.tile([P, 4 * PW], F16, name="gt")
                    for half in range(2):    # chunk pair (4gt+2*half, +1)
                        ps = gp.tile([P, 2, 512], F32, name="ps")
                        for jj in range(2):
                            mi = 4 * gt + 2 * half + jj
                            dst = ps[:, jj, :PW]
                            nc.tensor.matmul(
                                dst,
                                lhsT=q11d[:, :, mi * P : (mi + 1) * P],
                                rhs=xa[:],
                                start=True,
                                stop=False,
                                perf_mode=DRM,
                            )
                            nc.tensor.matmul(
                                dst,
                                lhsT=q11r[:, :, mi * P : (mi + 1) * P],
                                rhs=xb[:],
                                start=False,
                                stop=t0,
                                perf_mode=DRM,
                            )
                            if not t0:
                                for kb in range(2):
                                    nc.tensor.matmul(
                                        dst,
                                        lhsT=q1h[kb][:, :, mi * P : (mi + 1) * P],
                                        rhs=h1d[(t - 1) % 2][p_][:, 2 * kb * PW : (2 * kb + 2) * PW].rearrange("p (j n) -> p j n", j=2),
                                        start=False,
                                        stop=kb == 1,
                                        perf_mode=DRM,
                                    )
                        func = AFT.Tanh if gt == 2 else AFT.Sigmoid
                        nc.scalar.activation(
                            out=gw[:, 2 * half * PW : (2 * half + 2) * PW],
                            in_=ps[:, :, :PW],
                            func=func,
                            scale=DESC,
                        )
                    ga[gt] = gw
                return ga

            def l2_layer(p_, t):
                """Layer 2: fp8 DR (t<K8) or bf16 (t>=K8) matmuls + narrow
                activations carrying the b2 bias."""
                t0 = t == 0
                fp8 = t < K8
                ga = [None] * 4
                pairs = [(gt, half) for gt in range(4) for half in range(2)
                         if not (t0 and gt == 1)]
                for gt in range(4):
                    if not (t0 and gt == 1):
                        ga[gt] = gb.tile([P, 4 * PW], F16, name="gt")

                def emit_hh(ps, gt, half):
                    for jj in range(2):
                        mi = 4 * gt + 2 * half + jj
                        dst = ps[:, jj, :PW]
                        if fp8:
                            for kb in range(2):
                                nc.tensor.matmul(
                                    dst,
                                    lhsT=q2h[kb][:, :, mi * P : (mi + 1) * P],
                                    rhs=h2d[(t - 1) % 2][p_][:, 2 * kb * PW : (2 * kb + 2) * PW].rearrange("p (j n) -> p j n", j=2),
                                    start=kb == 0,
                                    stop=False,
                                    perf_mode=DRM,
                                )
                        else:
                            for k in range(4):
                                nc.tensor.matmul(
                                    dst,
                                    lhsT=w2hb[k][:, mi * P : (mi + 1) * P],
                                    rhs=h2b[p_][:, k * PW : (k + 1) * PW],
                                    start=k == 0,
                                    stop=False,
                                )

                def emit_ih_act(ps, gt, half):
                    for jj in range(2):
                        mi = 4 * gt + 2 * half + jj
                        dst = ps[:, jj, :PW]
                        first = t0
                        if fp8:
                            for kb in range(2):
                                nc.tensor.matmul(
                                    dst,
                                    lhsT=q2i[kb][:, :, mi * P : (mi + 1) * P],
                                    rhs=h1d[t % 2][p_][:, 2 * kb * PW : (2 * kb + 2) * PW].rearrange("p (j n) -> p j n", j=2),
                                    start=first,
                                    stop=kb == 1,
                                    perf_mode=DRM,
                                )
                                first = False
                        else:
                            h1w = h1b_cur[0]
                            for k in range(4):
                                nc.tensor.matmul(
                                    dst,
                                    lhsT=w2ib[k][:, mi * P : (mi + 1) * P],
                                    rhs=h1w[:, k * PW : (k + 1) * PW],
                                    start=first,
                                    stop=k == 3,
                                )
                                first = False
                    func = AFT.Tanh if gt == 2 else AFT.Sigmoid
                    gw = ga[gt]
                    for jj in range(2):
                        mi = 4 * gt + 2 * half + jj
                        nc.scalar.activation(
                            out=gw[:, (2 * half + jj) * PW : (2 * half + jj + 1) * PW],
                            in_=ps[:, jj, :PW],
                            func=func,
                            bias=b2_sb[:, mi : mi + 1],
                            scale=DESC,
                        )

                # software-pipeline the psum tiles (depth 3 = gp bufs):
                # emit hh-parts for up to 3 tiles before the first ih-part
                # so the h1 update chain has PE work to hide behind
                pend = []
                for gt, half in pairs:
                    ps = gp.tile([P, 2, 512], F32, name="ps")
                    if not t0:
                        emit_hh(ps, gt, half)
                    pend.append((ps, gt, half))
                    if len(pend) == 3:
                        emit_ih_act(*pend.pop(0))
                for ent in pend:
                    emit_ih_act(*ent)
                return ga

            def update(ga, c, t0, out8=None, outb=None, outf=None,
                       halves=False):
                """c update + h writes over [128, 4*PW] tiles. ga = [i, f,
                g, o]. out8: fp8 DR tile (written as SA*h); outb: bf16;
                outf: fp32. halves=True splits into kb-halves so the first
                half's h lands ~2.5us earlier (emission order avoids DVE
                FIFO head-blocking)."""
                gi, gf, gg, go = ga
                HW = 2 * PW
                cuts = [(0, HW), (HW, HW)] if halves else [(0, 4 * PW)]
                ths = []
                for o, w in cuts:
                    sl = slice(o, o + w)
                    if t0:
                        nc.vector.tensor_mul(out=c[:, sl], in0=gi[:, sl],
                                             in1=gg[:, sl])
                    else:
                        p1 = tp.tile([P, 4 * PW], F16, name="tpt")
                        nc.vector.tensor_mul(out=p1[:, sl], in0=gf[:, sl],
                                             in1=c[:, sl])
                        p2 = tp.tile([P, 4 * PW], F16, name="tpt")
                        nc.vector.tensor_mul(out=p2[:, sl], in0=gi[:, sl],
                                             in1=gg[:, sl])
                        nc.vector.tensor_add(out=c[:, sl], in0=p1[:, sl],
                                             in1=p2[:, sl])
                    th = tp.tile([P, 4 * PW], F16, name="tpt")
                    nc.scalar.activation(out=th[:, sl], in_=c[:, sl],
                                         func=AFT.Tanh)
                    ths.append(th)
                ndst = (out8 is not None) + (outb is not None) + (outf is not None)
                for (o, w), th in zip(cuts, ths):
                    sl = slice(o, o + w)
                    if ndst > 1:
                        hf = tp.tile([P, 4 * PW], F16, name="tpt")
                        nc.vector.tensor_mul(out=hf[:, sl], in0=go[:, sl],
                                             in1=th[:, sl])
                        if out8 is not None:
                            nc.vector.tensor_scalar_mul(out8[:, sl],
                                                        hf[:, sl], SA)
                        if outb is not None:
                            nc.vector.tensor_copy(out=outb[:, sl],
                                                  in_=hf[:, sl])
                        if outf is not None:
                            nc.vector.tensor_copy(out=outf[:, sl],
                                                  in_=hf[:, sl])
                    elif out8 is not None:
                        # (go * SA) * th -> fp8 in one fused op
                        nc.vector.scalar_tensor_tensor(
                            out=out8[:, sl], in0=go[:, sl], scalar=SA,
                            in1=th[:, sl],
                            op0=mybir.AluOpType.mult, op1=mybir.AluOpType.mult,
                        )
                    elif outb is not None:
                        nc.vector.tensor_mul(out=outb[:, sl], in0=go[:, sl],
                                             in1=th[:, sl])
                    else:
                        nc.vector.tensor_mul(out=outf[:, sl], in0=go[:, sl],
                                             in1=th[:, sl])

            NS = T * NPASS
            ests = {0: gen_gather(0, 0), 1: gen_gather(1, 1)}
            x_cur = gen_x(ests.pop(0), 0)
            h1b_cur = [None]
            for t in range(T):
                for p_ in range(NPASS):
                    s = t * NPASS + p_
                    wb = t % 2
                    t0 = t == 0
                    tail = t >= K8
                    g1 = l1_layer(x_cur, p_, t)
                    # h1 destinations: fp8 for next-step L1 and fp8-L2;
                    # bf16 for the bf16 L2 path
                    o8 = h1d[wb][p_] if (t < T - 1 or not tail) else None
                    if tail:
                        h1b_cur[0] = hb.tile([P, 4 * PW], BF16, name="h1b")
                    update(g1, c1[p_], t0, out8=o8,
                           outb=h1b_cur[0] if tail else None, halves=True)
                    # prefetch: gathers two slots ahead, transposes one
                    # slot ahead (fills the PE gap before L2's h1 wait)
                    if s + 2 < NS:
                        ests[s + 2] = gen_gather(s + 2, (s + 2) % 2)
                    if s + 1 < NS:
                        x_next = gen_x(ests.pop(s + 1), (s + 1) % 2)
                    else:
                        x_next = None
                    g2 = l2_layer(p_, t)
                    o8 = h2d[wb][p_] if t + 1 < K8 else None
                    ob = h2b[p_] if K8 - 1 <= t < T - 1 else None
                    if t == T - 1:
                        of = hb.tile([P, 4 * PW], F32, name="h2f")
                    else:
                        of = None
                    update(g2, c2[p_], t0, out8=o8, outb=ob, outf=of, halves=True)
                    if t == T - 1:
                        for r in range(4):
                            nc.sync.dma_start(
                                out=outd[r * P : (r + 1) * P, p_ * PW : (p_ + 1) * PW],
                                in_=of[:, r * PW : (r + 1) * PW],
                            )
                    x_cur = x_next
    _split_waits(nc)
    return nc


_NC_CACHE = None


def _get_nc():
    global _NC_CACHE
    if _NC_CACHE is None:
        _NC_CACHE = build_nc()
    return _NC_CACHE


def _sigmoid(x):
    return 1.0 / (1.0 + np.exp(-x))


def _gptq_e4m3(W, X, sc):
    """GPTQ-quantize W [M,K] to e4m3 at scale sc, calibrated on inputs
    X [N,K]. Returns the scaled quantized weights (fp32 values of sc*W)."""
    K = W.shape[1]
    H = (X.T @ X) / max(len(X), 1)
    H[np.diag_indices(K)] += 0.01 * np.mean(np.diag(H)) + 1e-8
    Hinv = np.linalg.inv(H)
    Wq = (W * sc).astype(np.float32).copy()
    Q = np.zeros_like(Wq)
    for j in range(K):
        q = Wq[:, j].astype(NP8).astype(np.float32)
        Q[:, j] = q
        err = (Wq[:, j] - q) / Hinv[j, j]
        if j + 1 < K:
            Wq[:, j + 1 :] -= np.outer(err, Hinv[j, j + 1 :])
    return Q


def _prep_core_inputs(sentence, word2vec, W_ih1, W_hh1, b_ih1, b_hh1,
                      W_ih2, W_hh2, b_ih2, b_hh2):
    f = lambda a: np.ascontiguousarray(np.asarray(a), dtype=np.float32)
    ids_all = np.asarray(sentence).reshape(BN, T).astype(np.int32)
    w2v = f(word2vec)
    w2v16 = np.tanh(w2v).astype(np.float16)
    W0 = {"W_ih1": f(W_ih1), "W_hh1": f(W_hh1),
          "W_ih2": f(W_ih2), "W_hh2": f(W_hh2)}
    b1 = f(b_ih1) + f(b_hh1)
    b2 = f(b_ih2) + f(b_hh2)

    # ---- bf16 weights (layer-2 late steps), pre-scaled x512 so the
    # PSUM scale matches the fp8 DR path ----
    fw = lambda a: np.ascontiguousarray((a.T * (SW * SA)).astype(NPBF))
    w2ib = fw(W0["W_ih2"])
    w2hb = fw(W0["W_hh2"])

    # ---- calibration run (CPU, fp32) for GPTQ Hessians ----
    CAL = 512
    cs = ids_all[:: max(BN // CAL, 1)][:CAL]
    h1 = np.zeros((CAL, RNN), np.float32)
    c1 = np.zeros_like(h1)
    h2 = np.zeros_like(h1)
    c2 = np.zeros_like(h1)
    Xx, Xh1, Xh1b, Xh2 = [], [], [], []
    for t in range(T):
        x = w2v16[cs[:, t]].astype(np.float32)
        Xx.append(x)
        Xh1.append(h1.copy())
        Xh2.append(h2.copy())
        g1 = x @ W0["W_ih1"].T + h1 @ W0["W_hh1"].T + b1
        i_, f_, g_, o_ = np.split(g1, 4, axis=1)
        c1 = _sigmoid(f_) * c1 + _sigmoid(i_) * np.tanh(g_)
        h1 = _sigmoid(o_) * np.tanh(c1)
        Xh1b.append(h1.copy())
        g2 = h1 @ W0["W_ih2"].T + h2 @ W0["W_hh2"].T + b2
        i_, f_, g_, o_ = np.split(g2, 4, axis=1)
        c2 = _sigmoid(f_) * c2 + _sigmoid(i_) * np.tanh(g_)
        h2 = _sigmoid(o_) * np.tanh(c2)

    Q = {}
    Q["W_ih1"] = _gptq_e4m3(W0["W_ih1"], np.concatenate(Xx) * SA, SW)
    Q["W_hh1"] = _gptq_e4m3(W0["W_hh1"], np.concatenate(Xh1) * SA, SW)
    Q["W_ih2"] = _gptq_e4m3(W0["W_ih2"], np.concatenate(Xh1b[:K8]) * SA, SW)
    Q["W_hh2"] = _gptq_e4m3(W0["W_hh2"], np.concatenate(Xh2[:K8]) * SA, SW)

    # ---- DR packs: tile[ki, j, m] = Q[m, 256*kb + 128*j + ki] ----
    def dr_pack(Qm, kb):
        lo = Qm[:, 256 * kb : 256 * kb + 128].T          # [128, M]
        hi = Qm[:, 256 * kb + 128 : 256 * kb + 256].T    # [128, M]
        return np.ascontiguousarray(
            np.stack([lo, hi], axis=1).astype(NP8))       # [128, 2, M]

    q11d = dr_pack(Q["W_ih1"], 0)
    # xb pack [23, 2, M]: rows p<22 = emb dims (256+p, 278+p); row 22 is
    # the bias (rhs value SA): j=0 main e4m3(64*b1), j=1 fp8 residual
    q11r = np.zeros((P, 2, 4 * RNN), np.float32)
    q11r[:22, 0, :] = Q["W_ih1"][:, 256:278].T
    q11r[:22, 1, :] = Q["W_ih1"][:, 278:300].T
    bmain = (SW * b1).astype(NP8).astype(np.float32)
    bres = (SW * b1 - bmain).astype(NP8).astype(np.float32)
    q11r[22, 0, :] = bmain
    q11r[22, 1, :] = bres
    q11r = np.ascontiguousarray(q11r.astype(NP8))
    q1h = np.stack([dr_pack(Q["W_hh1"], kb) for kb in range(2)])
    q2i = np.stack([dr_pack(Q["W_ih2"], kb) for kb in range(2)])
    q2h = np.stack([dr_pack(Q["W_hh2"], kb) for kb in range(2)])

    b2m = f(b2.reshape(NM, P).T)

    in_maps = []
    for k in range(NCORES):
        ids_k = ids_all[k * BNC : (k + 1) * BNC]
        ids_arr = np.zeros((P, P), dtype=np.int32)
        for t in range(T):
            for p_ in range(NPASS):
                s = t * NPASS + p_
                for j, (to, tn) in enumerate(TOKT):
                    g = s * len(TOKT) + j
                    ids_arr[:tn, g] = ids_k[p_ * PW + to : p_ * PW + to + tn, t]
        xbc = np.zeros((P, 2, PW), np.float32)
        xbc[22, :, :] = SA
        in_maps.append(
            {
                "xbc": xbc.astype(NP8),
                "w2v16": w2v16,
                "q11d": q11d,
                "q11r": q11r,
                "q1h": q1h,
                "q2i": q2i,
                "q2h": q2h,
                "w2ib": w2ib,
                "w2hb": w2hb,
                "b2": b2m,
                "ids": ids_arr,
            }
        )
    return in_maps


def kernel(sentence, word2vec, W_ih1, W_hh1, b_ih1, b_hh1,
           W_ih2, W_hh2, b_ih2, b_hh2, _trace=False, _return_perf=None):
    nc = _get_nc()
    in_maps = _prep_core_inputs(
        sentence, word2vec, W_ih1, W_hh1, b_ih1, b_hh1, W_ih2, W_hh2, b_ih2, b_hh2
    )
    res = run_bass_kernel_spmd(
        nc, in_maps, core_ids=list(range(NCORES)), trace=_trace
    )
    if _return_perf is not None:
        _return_perf.append(res)
    parts = [res.results[k]["out"].T for k in range(NCORES)]
    out = np.concatenate(parts, axis=0).reshape(B, NCLS, RNN)
    return np.ascontiguousarray(out, dtype=np.float32)
